# revision 18
# baseline (speedup 1.0000x reference)
"""DH-SFNN Trainium2 kernel (8 NeuronCores, data-parallel over batch).

Model: 2 dendritic LIF layers (K=4 branches, reset-by-subtraction) + leaky
readout integrator, T=250 steps, B=256, IN=700, H=256, O=20.

Fast path (per core, B_l=32), exploiting reset-by-subtraction soundness:
spike corrections are strictly subtractive, so if the no-spike layer-1
membrane trajectory m1^ satisfies max m1^ <= VTH there are exactly zero
layer-1 spikes. Layer 2 then sees only its bias trajectory (x-independent,
verified exactly on host), and the readout is a batch-independent constant
computed on host. The device therefore only needs to certify layer 1:

    c1 = x @ (16*W1).T (+bias row)     -- fp8 DoubleRow matmuls (2x128
                                          contraction rows per instr)
    d1 = per-feature 1-pole IIR over t -- DVE tensor_tensor_scan, 4 batch
                                          streams packed per instruction with
                                          zeroed-multiplier boundary columns
    D1 = sum_k g_k d1_k               -- PE matmul with g/16-weighted selector
    check max_t D1 <= VTH - 0.25      -- Act engine relu-accumulate; since
                                          m1^ is a running convex combination
                                          of D1, max m1^ <= max(0, max D1).

If the on-device flag fires, or the host-side layer-2 bias-trajectory check
fails, rerun with the general sequential-correction kernel (slow path).
"""
import sys

sys.path.insert(0, "/opt/trn_rl_repo")

import numpy as np
import ml_dtypes

import concourse.bass as bass
import concourse.mybir as mybir
import concourse.tile as tile
from concourse import bacc, bass_utils, bass_isa

F32 = mybir.dt.float32
BF16 = mybir.dt.bfloat16
FP8 = mybir.dt.float8e4
ALU = mybir.AluOpType
ACT = mybir.ActivationFunctionType
DR = mybir.MatmulPerfMode.DoubleRow

N_CORES = 8
B, T, IN, H, O, K = 256, 250, 700, 256, 20, 4
BL = B // N_CORES            # 32 batch per core
BBLK = 4                     # batches per scan slab
NBB = BL // BBLK             # 8 slabs
NSL = BBLK * T               # 1000 slab columns
IC = 6                       # 768 = 6*128 contraction rows (row 700 = bias)
NPR = IC // 2                # 3 DoubleRow pair chunks
NF = H * K                   # 1024 layer-1 branch features
NCF = NF // 128              # 8 feature chunks
VTH = 1.0
CHECK_MARGIN = 0.25          # device certifies max D <= VTH - margin
WSC = 16.0                   # power-of-2 prescale on W1 for fp8 range
# out-column splits of the 1000 slab columns, each within one PSUM bank
CSPLITS = [(0, 256), (256, 256), (512, 256), (768, 232)]
NN_SPLITS = [(0, 512), (512, 488)]


def _sig(v):
    return 1.0 / (1.0 + np.exp(-np.asarray(v, np.float64)))


def build_nc():
    nc = bacc.Bacc("TRN2", target_bir_lowering=False, debug=False,
                   num_devices=N_CORES)
    dt = nc.dram_tensor
    xq_d = dt("xq", [NPR, 128, 2, BL, T], FP8, kind="ExternalInput").ap()
    w1_d = dt("w1q", [NPR, 128, 2 * NF], FP8, kind="ExternalInput").ap()
    sel_d = dt("selm", [128, NCF * 32], BF16, kind="ExternalInput").ap()
    bsl_d = dt("bsl1", [NCF, 128, NSL], BF16, kind="ExternalInput").ap()
    outc_d = dt("outc", [O, BL], F32, kind="ExternalInput").ap()
    out_d = dt("out", [O, BL], F32, kind="ExternalOutput").ap()
    flag_d = dt("flag", [128, 1], F32, kind="ExternalOutput").ap()

    with tile.TileContext(nc) as tc:
        with tc.tile_pool(name="const", bufs=1) as cpool, \
             tc.tile_pool(name="xs", bufs=2) as xpool, \
             tc.tile_pool(name="ds", bufs=2) as dpool, \
             tc.tile_pool(name="small", bufs=1) as mpool:

            # ---- constants (issue order = SP issue order: PE deps first) ----
            w1sb = [cpool.tile([128, 2 * NF], FP8, name=f"w1sb{i}",
                               tag=f"w1_{i}") for i in range(NPR)]
            for i in range(NPR):
                nc.sync.dma_start(out=w1sb[i], in_=w1_d[i])
            bslsb = cpool.tile([128, NCF * NSL], BF16, name="bslsb")
            nc.sync.dma_start(out=bslsb[:, 0:NSL], in_=bsl_d[0])
            selsb = cpool.tile([128, NCF * 32], BF16, name="selsb")
            outcsb = cpool.tile([O, BL], F32, name="outcsb")
            biasc = mpool.tile([128, 1], F32, name="biasc")
            nc.vector.memset(biasc, -(VTH - CHECK_MARGIN))

            cnt = mpool.tile([128, 2 * NBB], F32, name="cnt")
            csum = mpool.tile([128, 1], F32, name="csum")
            junk = mpool.tile([128, NSL], BF16, name="junk")
            junk16 = mpool.tile([128, 2 * NBB], F32, name="junk16")

            with tc.tile_pool(name="psA", bufs=2, space="PSUM") as pspool, \
                 tc.tile_pool(name="psB", bufs=2, space="PSUM") as dppool:
                dss = {}

                def emit_x(bb, spread=False):
                    xs = []
                    eng = [nc.gpsimd] * NPR
                    if spread:
                        eng = [nc.sync, nc.scalar, nc.gpsimd]
                    for pr in range(NPR):
                        t_ = xpool.tile([128, 2 * NSL], FP8,
                                        name=f"xs{bb}_{pr}", tag=f"xs{pr}")
                        eng[pr].dma_start(
                            out=t_.rearrange("p (k b t) -> p k b t",
                                             k=2, b=BBLK),
                            in_=xq_d[pr][:, :, bb * BBLK:(bb + 1) * BBLK, :])
                        xs.append(t_.rearrange("p (k n) -> p k n", k=2))
                    return xs

                def emit_cmm_scan(bb, xs, cfs):
                    ds = dss[bb]
                    for cf in cfs:
                        ps = pspool.tile([128, 1024], F32,
                                         name=f"c{bb}_{cf}", tag="mm")
                        for n0, nw in CSPLITS:
                            for pr in range(NPR):
                                nc.tensor.matmul(
                                    ps[:, n0:n0 + nw],
                                    lhsT=w1sb[pr]
                                        .rearrange("p (k m) -> p k m", k=2)
                                        [:, :, cf * 128:(cf + 1) * 128],
                                    rhs=xs[pr][:, :, n0:n0 + nw],
                                    start=(pr == 0), stop=(pr == NPR - 1),
                                    perf_mode=DR)
                        nc.vector.tensor_tensor_scan(
                            out=ds[:, cf * NSL:(cf + 1) * NSL],
                            data0=bslsb[:, cf * NSL:(cf + 1) * NSL],
                            data1=ps[:, 0:NSL], initial=0.0,
                            op0=ALU.mult, op1=ALU.add)

                Dcur = {}

                def emit_sel_mm(bb, hh, c4s):
                    ds = dss[bb]
                    if (bb, hh) not in Dcur:
                        Dcur[(bb, hh)] = dppool.tile(
                            [128, 1024], F32, name=f"D{bb}_{hh}", tag="D")
                    Dps = Dcur[(bb, hh)]
                    for c4 in c4s:
                        cf = hh * 4 + c4
                        for n0, nw in NN_SPLITS:
                            nc.tensor.matmul(
                                Dps[c4 * 32:(c4 + 1) * 32, n0:n0 + nw],
                                lhsT=selsb[:, cf * 32:(cf + 1) * 32],
                                rhs=ds[:, cf * NSL + n0:cf * NSL + n0 + nw],
                                start=True, stop=True,
                                tile_position=(0, c4 * 32))

                def emit_check(bb, hh):
                    # spike certificate: relu(D - (VTH - margin)) summed
                    Dps = Dcur.pop((bb, hh))
                    nc.scalar.activation(
                        out=junk, in_=Dps[:, 0:NSL], func=ACT.Relu,
                        bias=biasc, scale=1.0,
                        accum_out=cnt[:, bb * 2 + hh:bb * 2 + hh + 1])

                def emit_sel_check(bb, hh):
                    emit_sel_mm(bb, hh, range(4))
                    emit_check(bb, hh)

                for bb in range(NBB):
                    dss[bb] = dpool.tile([128, NCF * NSL], BF16,
                                         name=f"ds{bb}", tag="ds")
                xs = emit_x(0, spread=True)
                # remaining constants issue behind the critical first slab
                nc.sync.dma_start(out=selsb, in_=sel_d)
                for i in range(1, NCF):
                    nc.sync.dma_start(out=bslsb[:, i * NSL:(i + 1) * NSL],
                                      in_=bsl_d[i])
                nc.sync.dma_start(out=outcsb, in_=outc_d)
                nc.sync.dma_start(out=out_d, in_=outcsb)
                last = NBB - 1
                for bb in range(NBB):
                    emit_cmm_scan(bb, xs, range(0, 4))
                    if bb > 0:
                        emit_sel_check(bb - 1, 0)
                    if bb < last:
                        emit_cmm_scan(bb, xs, range(4, NCF))
                        xs = emit_x(bb + 1)
                        if bb > 0:
                            emit_sel_check(bb - 1, 1)
                    else:
                        # final slab: chase each scan with its selector slice
                        emit_cmm_scan(bb, xs, [4])
                        emit_sel_check(bb - 1, 1)
                        emit_sel_mm(bb, 1, [0])
                        emit_sel_mm(bb, 0, range(4))
                        emit_check(bb, 0)
                        for cf in range(5, NCF):
                            emit_cmm_scan(bb, xs, [cf])
                            emit_sel_mm(bb, 1, [cf - 4])
                        emit_check(bb, 1)

            nc.scalar.activation(
                out=junk16, in_=cnt, func=ACT.Copy, bias=0.0, scale=1.0,
                accum_out=csum)
            nc.sync.dma_start(out=flag_d, in_=csum)

    nc.compile()
    return nc


# ---------------------------------------------------------------------------
# general fallback kernel (sequential spike-correction), used only when the
# no-spike certificate fails: identical to the reference recurrence.
# ---------------------------------------------------------------------------

def build_nc_slow():
    nc = bacc.Bacc("TRN2", target_bir_lowering=False, debug=False,
                   num_devices=N_CORES)
    dt = nc.dram_tensor
    xt_d = dt("xt", [IC * 128, BL, T], BF16, kind="ExternalInput").ap()
    w1_d = dt("w1p", [IC * 128, NF], BF16, kind="ExternalInput").ap()
    w2_d = dt("w2p", [H, NF], BF16, kind="ExternalInput").ap()
    wr_d = dt("wrt", [128, 2 * O], BF16, kind="ExternalInput").ap()
    m2b_d = dt("mh2b", [128, 2 * T], BF16, kind="ExternalInput").ap()
    bsl1_d = dt("bsl1", [NCF, 128, NSL], BF16, kind="ExternalInput").ap()
    bsl2_d = dt("bsl2", [NCF, 128, NSL], BF16, kind="ExternalInput").ap()
    asl_d = dt("asl", [128, 4 * NSL], BF16, kind="ExternalInput").ap()
    acol_d = dt("acol", [128, 4], F32, kind="ExternalInput").ap()
    sel_d = dt("selm", [128, 32], BF16, kind="ExternalInput").ap()
    ur_d = dt("ur", [O, T], F32, kind="ExternalInput").ap()
    bru_d = dt("bru", [O, 1], F32, kind="ExternalInput").ap()
    out_d = dt("out", [O, BL], F32, kind="ExternalOutput").ap()
    flag_d = dt("flag", [1, 2], F32, kind="ExternalOutput").ap()

    with tile.TileContext(nc) as tc:
        with tc.tile_pool(name="const", bufs=1) as cpool, \
             tc.tile_pool(name="state", bufs=1) as spool, \
             tc.tile_pool(name="bsl", bufs=1) as bpool, \
             tc.tile_pool(name="xs", bufs=2) as xpool, \
             tc.tile_pool(name="ds", bufs=2) as dpool, \
             tc.tile_pool(name="small", bufs=1) as mpool:

            w1sb = [cpool.tile([128, NF], BF16, name=f"w1sb{i}", tag=f"w1_{i}")
                    for i in range(IC)]
            for i in range(IC):
                nc.sync.dma_start(out=w1sb[i], in_=w1_d[i * 128:(i + 1) * 128, :])
            w2sb = [cpool.tile([128, NF], BF16, name=f"w2sb{i}", tag=f"w2_{i}")
                    for i in range(2)]
            for i in range(2):
                nc.sync.dma_start(out=w2sb[i], in_=w2_d[i * 128:(i + 1) * 128, :])
            wrsb = cpool.tile([128, 2 * O], BF16, name="wrsb")
            nc.sync.dma_start(out=wrsb, in_=wr_d)
            m2bsb = cpool.tile([128, 2 * T], BF16, name="m2bsb")
            nc.sync.dma_start(out=m2bsb, in_=m2b_d)
            aslsb = cpool.tile([128, 4 * NSL], BF16, name="aslsb")
            nc.sync.dma_start(out=aslsb, in_=asl_d)
            acolsb = cpool.tile([128, 4], F32, name="acolsb")
            nc.sync.dma_start(out=acolsb, in_=acol_d)
            selsb = cpool.tile([128, 32], BF16, name="selsb")
            nc.sync.dma_start(out=selsb, in_=sel_d)
            ursb = cpool.tile([O, T], F32, name="ursb")
            nc.sync.dma_start(out=ursb, in_=ur_d)
            brusb = cpool.tile([O, 1], F32, name="brusb")
            nc.sync.dma_start(out=brusb, in_=bru_d)

            mhat = spool.tile([128, 2 * NBB * NSL], BF16, name="mhat")
            sfull = spool.tile([128, 2 * NBB * NSL], BF16, name="sfull")
            q = mpool.tile([128, 64], BF16, name="q")
            cnt = mpool.tile([128, 4], F32, name="cnt")
            csum = mpool.tile([128, 2], F32, name="csum")
            par = mpool.tile([128, 2], F32, name="par")
            acc = mpool.tile([O, BL], F32, name="acc")
            accb = mpool.tile([O, BL], F32, name="accb")
            zjunk = mpool.tile([O, T], F32, name="zjunk")

            mh_v = mhat.rearrange("p (hh b t) -> p hh b t", hh=2, b=BL, t=T)
            sf_v = sfull.rearrange("p (hh b t) -> p hh b t", hh=2, b=BL, t=T)
            q_v = q.rearrange("p (hh b) -> p hh b", hh=2)

            with tc.tile_pool(name="psA", bufs=2, space="PSUM") as pspool:

                def layer(L, bsl_d, rhs_mm):
                    bslsb = bpool.tile([128, NCF * NSL], BF16, name=f"bslsb{L}",
                                       tag="bsl")
                    for cf in range(NCF):
                        nc.sync.dma_start(out=bslsb[:, cf * NSL:(cf + 1) * NSL],
                                          in_=bsl_d[cf])
                    aoff = (L - 1) * 2 * NSL
                    for bb in range(NBB):
                        ds = dpool.tile([128, NCF * NSL], BF16,
                                        name=f"ds{L}_{bb}", tag="ds")
                        for cf in range(NCF):
                            ps = pspool.tile([128, NSL], F32,
                                             name=f"c{L}_{bb}_{cf}", tag="mm")
                            for nn in range(2):
                                rhs_mm(ps, bb, cf, nn)
                            nc.vector.tensor_tensor_scan(
                                out=ds[:, cf * NSL:(cf + 1) * NSL],
                                data0=bslsb[:, cf * NSL:(cf + 1) * NSL],
                                data1=ps,
                                initial=0.0, op0=ALU.mult, op1=ALU.add)
                        for hh in range(2):
                            Dps = pspool.tile([128, 1024], F32,
                                              name=f"D{L}_{bb}_{hh}", tag="D")
                            for c4 in range(4):
                                o4 = (hh * 4 + c4) * NSL
                                for n0, nw in NN_SPLITS:
                                    nc.tensor.matmul(
                                        Dps[c4 * 32:(c4 + 1) * 32,
                                            n0:n0 + nw],
                                        lhsT=selsb,
                                        rhs=ds[:, o4 + n0:o4 + n0 + nw],
                                        start=True, stop=True,
                                        tile_position=(0, c4 * 32))
                            nc.vector.tensor_tensor_scan(
                                out=mhat[:, hh * 8000 + bb * NSL:
                                         hh * 8000 + (bb + 1) * NSL],
                                data0=aslsb[:, aoff + hh * NSL:
                                            aoff + (hh + 1) * NSL],
                                data1=Dps[:, 0:NSL], initial=0.0,
                                op0=ALU.mult, op1=ALU.add)

                def spike_phase(L):
                    nc.gpsimd.memset(sfull, 0.0)
                    junk = dpool.tile([128, NCF * NSL], BF16,
                                      name=f"junk{L}", tag="ds")
                    for hh in range(2):
                        nc.vector.tensor_scalar(
                            out=junk[:, 0:8000],
                            in0=mhat[:, hh * 8000:(hh + 1) * 8000],
                            scalar1=float(VTH), scalar2=None, op0=ALU.is_gt,
                            op1=ALU.add,
                            accum_out=cnt[:, (L - 1) * 2 + hh:(L - 1) * 2 + hh + 1])
                    nc.vector.tensor_add(
                        out=csum[:, L - 1:L],
                        in0=cnt[:, (L - 1) * 2:(L - 1) * 2 + 1],
                        in1=cnt[:, (L - 1) * 2 + 1:(L - 1) * 2 + 2])
                    nc.gpsimd.partition_all_reduce(
                        par[:, L - 1:L], csum[:, L - 1:L], channels=128,
                        reduce_op=bass_isa.ReduceOp.add)
                    nc.vector.memset(q, 0.0)
                    for t in range(T):
                        nc.vector.scalar_tensor_tensor(
                            out=sf_v[:, :, :, t], in0=mh_v[:, :, :, t],
                            scalar=float(VTH), op0=ALU.subtract,
                            in1=q_v, op1=ALU.is_gt)
                        for hh in range(2):
                            nc.vector.scalar_tensor_tensor(
                                out=q[:, hh * 32:(hh + 1) * 32],
                                in0=q[:, hh * 32:(hh + 1) * 32],
                                scalar=acolsb[:, (L - 1) * 2 + hh:
                                              (L - 1) * 2 + hh + 1],
                                op0=ALU.mult,
                                in1=sf_v[:, hh, :, t], op1=ALU.add)

                xs = {}

                def mm1(ps, bb, cf, nn):
                    n0, nw = NN_SPLITS[nn]
                    if cf == 0 and nn == 0:
                        for i in range(IC):
                            t_ = xpool.tile([128, NSL], BF16,
                                            name=f"xs{bb}_{i}", tag=f"xs{i}")
                            nc.sync.dma_start(
                                out=t_.rearrange("p (b t) -> p b t", b=BBLK),
                                in_=xt_d[i * 128:(i + 1) * 128,
                                         bb * BBLK:(bb + 1) * BBLK, :])
                            xs[i] = t_
                    for i in range(IC):
                        nc.tensor.matmul(
                            ps[:, n0:n0 + nw],
                            lhsT=w1sb[i][:, cf * 128:(cf + 1) * 128],
                            rhs=xs[i][:, n0:n0 + nw],
                            start=(i == 0), stop=(i == IC - 1))

                layer(1, bsl1_d, mm1)
                spike_phase(1)

                def mm2(ps, bb, cf, nn):
                    n0, nw = NN_SPLITS[nn]
                    for hh in range(2):
                        nc.tensor.matmul(
                            ps[:, n0:n0 + nw],
                            lhsT=w2sb[hh][:, cf * 128:(cf + 1) * 128],
                            rhs=sfull[:, hh * 8000 + bb * NSL + n0:
                                      hh * 8000 + bb * NSL + n0 + nw],
                            start=(hh == 0), stop=(hh == 1))

                layer(2, bsl2_d, mm2)
                nc.vector.tensor_add(
                    out=mh_v, in0=mh_v,
                    in1=m2bsb.rearrange("p (hh t) -> p hh t", hh=2)
                        .unsqueeze(2).broadcast_to((128, 2, BL, T)))
                spike_phase(2)

            with tc.tile_pool(name="psB", bufs=2, space="PSUM") as zpool:
                for bb in range(NBB):
                    for nn in range(2):
                        zps = zpool.tile([O, 500], F32, name=f"z{bb}_{nn}",
                                         tag="z")
                        for hh in range(2):
                            nc.tensor.matmul(
                                zps,
                                lhsT=wrsb[:, hh * O:(hh + 1) * O],
                                rhs=sfull[:, hh * 8000 + bb * NSL + nn * 500:
                                          hh * 8000 + bb * NSL + (nn + 1) * 500],
                                start=(hh == 0), stop=(hh == 1))
                        for b2 in range(2):
                            b = bb * BBLK + nn * 2 + b2
                            nc.vector.scalar_tensor_tensor(
                                out=zjunk, in0=zps[:, b2 * T:(b2 + 1) * T],
                                scalar=1.0, op0=ALU.mult,
                                in1=ursb, op1=ALU.mult,
                                accum_out=acc[:, b:b + 1])
                nc.vector.tensor_scalar(
                    out=accb, in0=acc, scalar1=brusb[:, 0:1], scalar2=None,
                    op0=ALU.add)
                nc.sync.dma_start(out=out_d, in_=accb)
                nc.sync.dma_start(out=flag_d, in_=par[0:1, 0:2])

    nc.compile()
    return nc


_NC_CACHE = {}


def get_nc():
    if "fast" not in _NC_CACHE:
        _NC_CACHE["fast"] = build_nc()
    return _NC_CACHE["fast"]


def get_nc_slow():
    if "slow" not in _NC_CACHE:
        _NC_CACHE["slow"] = build_nc_slow()
    return _NC_CACHE["slow"]


def prep_inputs(x, W1, b1, tau_n1, tau_m1, W2, b2, tau_n2, tau_m2,
                Wr, br, tau_mr, warmup):
    """Host-side: per-core input dicts for the fast bass kernel, plus the
    host-verified layer-2/readout constants. Returns (in_maps, fast_ok)."""
    w = int(np.asarray(warmup))
    beta1 = _sig(tau_n1).reshape(NF)          # [H,K], j = h*4+k order
    alpha1 = _sig(tau_m1)                     # [H]
    beta2 = _sig(tau_n2).reshape(NF)
    alpha2 = _sig(tau_m2)
    alphar = _sig(tau_mr)                     # [O]

    g1 = (1.0 - beta1) * np.repeat(1.0 - alpha1, K)

    # fp8 weights, prescaled by WSC; row 700 = bias, rows 701.. = 0
    w1t = np.zeros((IC * 128, NF), np.float64)
    w1t[:IN] = np.asarray(W1, np.float64).T * WSC
    w1t[IN] = np.asarray(b1, np.float64) * WSC
    w1q = np.empty((NPR, 128, 2 * NF), ml_dtypes.float8_e4m3)
    for pr in range(NPR):
        w1q[pr, :, :NF] = w1t[2 * pr * 128:(2 * pr + 1) * 128]
        w1q[pr, :, NF:] = w1t[(2 * pr + 1) * 128:(2 * pr + 2) * 128]

    # selector: g/WSC weights, [128, 32] blocks per feature chunk, packed
    selm = np.zeros((128, NCF * 32), ml_dtypes.bfloat16)
    for cf in range(NCF):
        j = cf * 128 + np.arange(128)
        selm[np.arange(128), cf * 32 + np.arange(128) // 4] = g1[j] / WSC

    def bslab(beta):
        s = np.tile(beta.reshape(NCF, 128, 1).astype(ml_dtypes.bfloat16),
                    (1, 1, NSL))
        s.reshape(NCF, 128, BBLK, T)[:, :, :, 0] = 0.0
        return s

    bsl1 = bslab(beta1)

    # host-exact layer-2 bias trajectory (valid when layer 1 has no spikes)
    b2g = np.asarray(b2, np.float64) * (1.0 - beta2)
    dtraj = np.zeros(NF)
    mtraj = np.zeros(H)
    m2max = -np.inf
    for _ in range(T):
        dtraj = beta2 * dtraj + b2g
        mtraj = alpha2 * mtraj + (1.0 - alpha2) * dtraj.reshape(H, K).sum(-1)
        m2max = max(m2max, mtraj.max())
    fast_ok = bool(m2max <= VTH - 0.05)

    # host-exact readout constant (valid when layer 2 has no spikes)
    mr = np.zeros(O)
    accr = np.zeros(O)
    for t_ in range(T):
        mr = mr * alphar + (1.0 - alphar) * np.asarray(br, np.float64)
        if t_ >= w:
            accr += mr
    outc = np.tile((accr / (T - w)).astype(np.float32)[:, None], (1, BL))

    xq_full = np.zeros((IC * 128, B, T), ml_dtypes.float8_e4m3)
    xq_full[:IN] = np.asarray(x).transpose(2, 0, 1)
    xq_full[IN] = 1.0
    # pair-interleaved: [NPR, 128, 2, B, T]
    xq_full = np.ascontiguousarray(
        xq_full.reshape(NPR, 2, 128, B, T).transpose(0, 2, 1, 3, 4))

    shared = dict(w1q=w1q, selm=selm, bsl1=bsl1, outc=outc)
    in_maps = []
    for c in range(N_CORES):
        m = dict(shared)
        m["xq"] = np.ascontiguousarray(
            xq_full[:, :, :, c * BL:(c + 1) * BL, :])
        in_maps.append(m)
    return in_maps, fast_ok


def prep_inputs_slow(x, W1, b1, tau_n1, tau_m1, W2, b2, tau_n2, tau_m2,
                     Wr, br, tau_mr, warmup):
    """Host-side prep for the general fallback kernel."""
    w = int(np.asarray(warmup))
    beta1 = _sig(tau_n1).reshape(NF)
    alpha1 = _sig(tau_m1)
    beta2 = _sig(tau_n2).reshape(NF)
    alpha2 = _sig(tau_m2)
    alphar = _sig(tau_mr)

    g1 = (1.0 - beta1) * np.repeat(1.0 - alpha1, K)
    g2 = (1.0 - beta2) * np.repeat(1.0 - alpha2, K)

    w1p = np.zeros((IC * 128, NF), np.float64)
    w1p[:IN] = np.asarray(W1, np.float64).T * g1
    w1p[IN] = np.asarray(b1, np.float64) * g1
    w1p = w1p.astype(ml_dtypes.bfloat16)

    w2p = (np.asarray(W2, np.float64).T * g2).astype(ml_dtypes.bfloat16)
    b2g = np.asarray(b2, np.float64) * g2
    dtraj = np.zeros(NF)
    mh2b = np.zeros((H, T))
    mtraj = np.zeros(H)
    for t_ in range(T):
        dtraj = _sig(tau_n2).reshape(NF) * dtraj + b2g
        mtraj = _sig(tau_m2) * mtraj + dtraj.reshape(H, K).sum(-1)
        mh2b[:, t_] = mtraj
    mh2b_dev = np.zeros((128, 2 * T), np.float64)
    mh2b_dev[:, :T] = mh2b[:128]
    mh2b_dev[:, T:] = mh2b[128:]
    mh2b_dev = mh2b_dev.astype(ml_dtypes.bfloat16)

    wrt = np.zeros((128, 2 * O), np.float64)
    wrt[:, :O] = np.asarray(Wr, np.float64).T[:128]
    wrt[:, O:] = np.asarray(Wr, np.float64).T[128:]
    wrt = wrt.astype(ml_dtypes.bfloat16)

    def bslab(beta):
        s = np.tile(beta.reshape(NCF, 128, 1).astype(ml_dtypes.bfloat16),
                    (1, 1, NSL))
        s.reshape(NCF, 128, BBLK, T)[:, :, :, 0] = 0.0
        return s

    bsl1 = bslab(beta1)
    bsl2 = bslab(beta2)

    def aslab(alpha):
        a2 = alpha.reshape(2, 128).astype(ml_dtypes.bfloat16)
        s = np.tile(a2[:, :, None], (1, 1, NSL))
        s.reshape(2, 128, BBLK, T)[:, :, :, 0] = 0.0
        return s

    asl = np.concatenate([aslab(alpha1), aslab(alpha2)], axis=0)
    asl = asl.transpose(1, 0, 2).reshape(128, 4 * NSL).copy()

    acol = np.stack([alpha1[:128], alpha1[128:], alpha2[:128], alpha2[128:]],
                    axis=1).astype(np.float32)

    selm = np.zeros((128, 32), ml_dtypes.bfloat16)
    selm[np.arange(128), np.arange(128) // 4] = 1.0

    tt = np.arange(T, dtype=np.float64)[:, None]
    ar = alphar[None, :]
    u = ar ** np.maximum(0, w - tt) - ar ** (T - tt)
    ur = (u.T / (T - w)).astype(np.float32)
    bru = (np.asarray(br, np.float64) * u.sum(0) / (T - w)) \
        .astype(np.float32)[:, None]

    xt_full = np.zeros((IC * 128, B, T), ml_dtypes.bfloat16)
    xt_full[:IN] = np.asarray(x).transpose(2, 0, 1)
    xt_full[IN] = 1.0

    shared = dict(w1p=w1p, w2p=w2p, mh2b=mh2b_dev, wrt=wrt,
                  bsl1=bsl1, bsl2=bsl2, asl=asl, acol=acol, selm=selm,
                  ur=ur, bru=bru)
    in_maps = []
    for c in range(N_CORES):
        m = dict(shared)
        m["xt"] = np.ascontiguousarray(xt_full[:, c * BL:(c + 1) * BL, :])
        in_maps.append(m)
    return in_maps


def _run_slow(**inputs):
    in_maps = prep_inputs_slow(**inputs)
    res = bass_utils.run_bass_kernel_spmd(
        get_nc_slow(), in_maps, core_ids=list(range(N_CORES)))
    out = np.empty((B, O), np.float32)
    for c in range(N_CORES):
        out[c * BL:(c + 1) * BL] = res.results[c]["out"].T
    return out


def kernel(**inputs):
    in_maps, fast_ok = prep_inputs(**inputs)
    if not fast_ok:
        return _run_slow(**inputs)
    res = bass_utils.run_bass_kernel_spmd(
        get_nc(), in_maps, core_ids=list(range(N_CORES)))
    if any(r["flag"].sum() > 0 for r in res.results):
        # certificate failed: spikes may exist, use the general kernel
        return _run_slow(**inputs)
    out = np.empty((B, O), np.float32)
    for c in range(N_CORES):
        out[c * BL:(c + 1) * BL] = res.results[c]["out"].T
    return out


# revision 23
# speedup vs baseline: 1.0264x; 1.0264x over previous
"""DH-SFNN Trainium2 kernel (8 NeuronCores, data-parallel over batch).

Model: 2 dendritic LIF layers (K=4 branches, reset-by-subtraction) + leaky
readout integrator, T=250 steps, B=256, IN=700, H=256, O=20.

Fast path (per core, B_l=32), exploiting reset-by-subtraction soundness:
spike corrections are strictly subtractive, so if the no-spike layer-1
membrane trajectory m1^ satisfies max m1^ <= VTH there are exactly zero
layer-1 spikes. Layer 2 then sees only its bias trajectory (x-independent,
verified exactly on host), and the readout is a batch-independent constant
computed on host. The device therefore only needs to certify layer 1:

    c1 = x @ (16*W1).T (+bias row)     -- fp8 DoubleRow matmuls (2x128
                                          contraction rows per instr)
    d1 = per-feature 1-pole IIR over t -- DVE tensor_tensor_scan, 4 batch
                                          streams packed per instruction with
                                          zeroed-multiplier boundary columns
    D1 = sum_k g_k d1_k               -- PE matmul with g/16-weighted selector
    check max_t D1 <= VTH - 0.25      -- Act engine relu-accumulate; since
                                          m1^ is a running convex combination
                                          of D1, max m1^ <= max(0, max D1).

If the on-device flag fires, or the host-side layer-2 bias-trajectory check
fails, rerun with the general sequential-correction kernel (slow path).
"""
import sys

sys.path.insert(0, "/opt/trn_rl_repo")

import numpy as np
import ml_dtypes

import concourse.bass as bass
import concourse.mybir as mybir
import concourse.tile as tile
from concourse import bacc, bass_utils, bass_isa

F32 = mybir.dt.float32
BF16 = mybir.dt.bfloat16
FP8 = mybir.dt.float8e4
ALU = mybir.AluOpType
ACT = mybir.ActivationFunctionType
DR = mybir.MatmulPerfMode.DoubleRow

N_CORES = 8
B, T, IN, H, O, K = 256, 250, 700, 256, 20, 4
BL = B // N_CORES            # 32 batch per core
BBLK = 4                     # batches per scan slab
NBB = BL // BBLK             # 8 slabs
NSL = BBLK * T               # 1000 slab columns
IC = 6                       # 768 = 6*128 contraction rows (row 700 = bias)
NPR = IC // 2                # 3 DoubleRow pair chunks
NF = H * K                   # 1024 layer-1 branch features
NCF = NF // 128              # 8 feature chunks
VTH = 1.0
CHECK_MARGIN = 0.25          # device certifies max D <= VTH - margin
WSC = 16.0                   # power-of-2 prescale on W1 for fp8 range
# out-column splits of the 1000 slab columns, each within one PSUM bank
CSPLITS = [(0, 256), (256, 256), (512, 256), (768, 232)]
NN_SPLITS = [(0, 512), (512, 488)]


def _sig(v):
    return 1.0 / (1.0 + np.exp(-np.asarray(v, np.float64)))


def build_nc():
    nc = bacc.Bacc("TRN2", target_bir_lowering=False, debug=False,
                   num_devices=N_CORES)
    dt = nc.dram_tensor
    xq_d = dt("xq", [NPR, 128, 2, BL, T], FP8, kind="ExternalInput").ap()
    w1_d = dt("w1q", [NPR, 128, 2 * NF], FP8, kind="ExternalInput").ap()
    sel_d = dt("selm", [128, NCF * 32], BF16, kind="ExternalInput").ap()
    bsl_d = dt("bsl1", [NCF, 128, NSL], BF16, kind="ExternalInput").ap()
    outc_d = dt("outc", [O, BL], F32, kind="ExternalInput").ap()
    out_d = dt("out", [O, BL], F32, kind="ExternalOutput").ap()
    flag_d = dt("flag", [128, 1], F32, kind="ExternalOutput").ap()

    with tile.TileContext(nc) as tc:
        with tc.tile_pool(name="const", bufs=1) as cpool, \
             tc.tile_pool(name="xs", bufs=2) as xpool, \
             tc.tile_pool(name="ds", bufs=2) as dpool, \
             tc.tile_pool(name="small", bufs=1) as mpool:

            # ---- constants (wire order: w1-0, bsl0, x0, w1-1/2 first) ----
            w1sb = [cpool.tile([128, 2 * NF], FP8, name=f"w1sb{i}",
                               tag=f"w1_{i}") for i in range(NPR)]
            bslsb = cpool.tile([128, NCF * NSL], BF16, name="bslsb")
            nc.sync.dma_start(out=w1sb[0], in_=w1_d[0])
            nc.sync.dma_start(out=bslsb[:, 0:NSL], in_=bsl_d[0])
            nc.sync.dma_start(out=w1sb[1], in_=w1_d[1])
            nc.sync.dma_start(out=w1sb[2], in_=w1_d[2])
            selsb = cpool.tile([128, NCF * 32], BF16, name="selsb")
            outcsb = cpool.tile([O, BL], F32, name="outcsb")
            biasc = mpool.tile([128, 1], F32, name="biasc")
            nc.vector.memset(biasc, -(VTH - CHECK_MARGIN))
            # PE p-state warmup: dummy matmuls on a zeroed scratch tile keep
            # the tensor engine ramping while the first DMAs land.
            wscr = mpool.tile([128, 128], BF16, name="wscr")
            nc.vector.memset(wscr, 0.0)

            cnt = mpool.tile([128, 2 * NBB], F32, name="cnt")
            csum = mpool.tile([128, 1], F32, name="csum")
            junk = mpool.tile([128, NSL], BF16, name="junk")
            junk16 = mpool.tile([128, 2 * NBB], F32, name="junk16")

            with tc.tile_pool(name="psA", bufs=2, space="PSUM") as pspool, \
                 tc.tile_pool(name="psB", bufs=2, space="PSUM") as dppool:
                dss = {}

                def emit_x(bb, spread=False):
                    xs = []
                    eng = [nc.gpsimd] * NPR
                    if spread:
                        eng = [nc.sync, nc.scalar, nc.gpsimd]
                    for pr in range(NPR):
                        t_ = xpool.tile([128, 2 * NSL], FP8,
                                        name=f"xs{bb}_{pr}", tag=f"xs{pr}")
                        eng[pr].dma_start(
                            out=t_.rearrange("p (k b t) -> p k b t",
                                             k=2, b=BBLK),
                            in_=xq_d[pr][:, :, bb * BBLK:(bb + 1) * BBLK, :])
                        xs.append(t_.rearrange("p (k n) -> p k n", k=2))
                    return xs

                def emit_cmm_scan(bb, xs, cfs, pre_cf=None):
                    ds = dss[bb]
                    for cf in cfs:
                        if pre_cf is not None:
                            pre_cf(cf)
                        ps = pspool.tile([128, 1024], F32,
                                         name=f"c{bb}_{cf}", tag="mm")
                        for n0, nw in CSPLITS:
                            for pr in range(NPR):
                                nc.tensor.matmul(
                                    ps[:, n0:n0 + nw],
                                    lhsT=w1sb[pr]
                                        .rearrange("p (k m) -> p k m", k=2)
                                        [:, :, cf * 128:(cf + 1) * 128],
                                    rhs=xs[pr][:, :, n0:n0 + nw],
                                    start=(pr == 0), stop=(pr == NPR - 1),
                                    perf_mode=DR)
                        nc.vector.tensor_tensor_scan(
                            out=ds[:, cf * NSL:(cf + 1) * NSL],
                            data0=bslsb[:, cf * NSL:(cf + 1) * NSL],
                            data1=ps[:, 0:NSL], initial=0.0,
                            op0=ALU.mult, op1=ALU.add)

                Dcur = {}

                def emit_sel_mm(bb, hh, c4s):
                    ds = dss[bb]
                    if (bb, hh) not in Dcur:
                        Dcur[(bb, hh)] = dppool.tile(
                            [128, 1024], F32, name=f"D{bb}_{hh}", tag="D")
                    Dps = Dcur[(bb, hh)]
                    for c4 in c4s:
                        cf = hh * 4 + c4
                        for n0, nw in NN_SPLITS:
                            nc.tensor.matmul(
                                Dps[c4 * 32:(c4 + 1) * 32, n0:n0 + nw],
                                lhsT=selsb[:, cf * 32:(cf + 1) * 32],
                                rhs=ds[:, cf * NSL + n0:cf * NSL + n0 + nw],
                                start=True, stop=True,
                                tile_position=(0, c4 * 32))

                def emit_check(bb, hh):
                    # spike certificate: relu(D - (VTH - margin)) summed
                    Dps = Dcur.pop((bb, hh))
                    nc.scalar.activation(
                        out=junk, in_=Dps[:, 0:NSL], func=ACT.Relu,
                        bias=biasc, scale=1.0,
                        accum_out=cnt[:, bb * 2 + hh:bb * 2 + hh + 1])

                def emit_sel_check(bb, hh):
                    emit_sel_mm(bb, hh, range(4))
                    emit_check(bb, hh)

                for bb in range(NBB):
                    dss[bb] = dpool.tile([128, NCF * NSL], BF16,
                                         name=f"ds{bb}", tag="ds")
                xs = emit_x(0, spread=True)
                # warmup: ~3us of dummy PE work to reach full clock before
                # the first real matmul
                wps = pspool.tile([128, 1024], F32, name="warm", tag="mm")
                for i in range(30):
                    nc.tensor.matmul(wps[0:128, 0:128], lhsT=wscr, rhs=wscr,
                                     start=True, stop=True)

                def bsl_drip(cf):
                    # slab 0: each beta-slab slice lands just before its scan
                    if cf > 0:
                        nc.sync.dma_start(
                            out=bslsb[:, cf * NSL:(cf + 1) * NSL],
                            in_=bsl_d[cf])
                    if cf == 4:
                        nc.sync.dma_start(out=selsb, in_=sel_d)

                last = NBB - 1
                for bb in range(NBB):
                    emit_cmm_scan(bb, xs, range(0, 4),
                                  pre_cf=bsl_drip if bb == 0 else None)
                    if bb > 0:
                        emit_sel_check(bb - 1, 0)
                    if bb < last:
                        emit_cmm_scan(bb, xs, range(4, NCF),
                                      pre_cf=bsl_drip if bb == 0 else None)
                        xs = emit_x(bb + 1)
                        if bb == 0:
                            nc.sync.dma_start(out=outcsb, in_=outc_d)
                            nc.sync.dma_start(out=out_d, in_=outcsb)
                        if bb > 0:
                            emit_sel_check(bb - 1, 1)
                    else:
                        # final slab: emit matmuls ahead so DVE stays fed,
                        # then chase the last scans with small selector slices
                        emit_cmm_scan(bb, xs, range(4, NCF))
                        emit_sel_check(bb - 1, 1)
                        emit_sel_mm(bb, 0, range(4))
                        emit_check(bb, 0)
                        for c4 in range(4):
                            emit_sel_mm(bb, 1, [c4])
                        emit_check(bb, 1)

            nc.scalar.activation(
                out=junk16, in_=cnt, func=ACT.Copy, bias=0.0, scale=1.0,
                accum_out=csum)
            nc.sync.dma_start(out=flag_d, in_=csum)

    nc.compile()
    return nc


# ---------------------------------------------------------------------------
# general fallback kernel (sequential spike-correction), used only when the
# no-spike certificate fails: identical to the reference recurrence.
# ---------------------------------------------------------------------------

def build_nc_slow():
    nc = bacc.Bacc("TRN2", target_bir_lowering=False, debug=False,
                   num_devices=N_CORES)
    dt = nc.dram_tensor
    xt_d = dt("xt", [IC * 128, BL, T], BF16, kind="ExternalInput").ap()
    w1_d = dt("w1p", [IC * 128, NF], BF16, kind="ExternalInput").ap()
    w2_d = dt("w2p", [H, NF], BF16, kind="ExternalInput").ap()
    wr_d = dt("wrt", [128, 2 * O], BF16, kind="ExternalInput").ap()
    m2b_d = dt("mh2b", [128, 2 * T], BF16, kind="ExternalInput").ap()
    bsl1_d = dt("bsl1", [NCF, 128, NSL], BF16, kind="ExternalInput").ap()
    bsl2_d = dt("bsl2", [NCF, 128, NSL], BF16, kind="ExternalInput").ap()
    asl_d = dt("asl", [128, 4 * NSL], BF16, kind="ExternalInput").ap()
    acol_d = dt("acol", [128, 4], F32, kind="ExternalInput").ap()
    sel_d = dt("selm", [128, 32], BF16, kind="ExternalInput").ap()
    ur_d = dt("ur", [O, T], F32, kind="ExternalInput").ap()
    bru_d = dt("bru", [O, 1], F32, kind="ExternalInput").ap()
    out_d = dt("out", [O, BL], F32, kind="ExternalOutput").ap()
    flag_d = dt("flag", [1, 2], F32, kind="ExternalOutput").ap()

    with tile.TileContext(nc) as tc:
        with tc.tile_pool(name="const", bufs=1) as cpool, \
             tc.tile_pool(name="state", bufs=1) as spool, \
             tc.tile_pool(name="bsl", bufs=1) as bpool, \
             tc.tile_pool(name="xs", bufs=2) as xpool, \
             tc.tile_pool(name="ds", bufs=2) as dpool, \
             tc.tile_pool(name="small", bufs=1) as mpool:

            w1sb = [cpool.tile([128, NF], BF16, name=f"w1sb{i}", tag=f"w1_{i}")
                    for i in range(IC)]
            for i in range(IC):
                nc.sync.dma_start(out=w1sb[i], in_=w1_d[i * 128:(i + 1) * 128, :])
            w2sb = [cpool.tile([128, NF], BF16, name=f"w2sb{i}", tag=f"w2_{i}")
                    for i in range(2)]
            for i in range(2):
                nc.sync.dma_start(out=w2sb[i], in_=w2_d[i * 128:(i + 1) * 128, :])
            wrsb = cpool.tile([128, 2 * O], BF16, name="wrsb")
            nc.sync.dma_start(out=wrsb, in_=wr_d)
            m2bsb = cpool.tile([128, 2 * T], BF16, name="m2bsb")
            nc.sync.dma_start(out=m2bsb, in_=m2b_d)
            aslsb = cpool.tile([128, 4 * NSL], BF16, name="aslsb")
            nc.sync.dma_start(out=aslsb, in_=asl_d)
            acolsb = cpool.tile([128, 4], F32, name="acolsb")
            nc.sync.dma_start(out=acolsb, in_=acol_d)
            selsb = cpool.tile([128, 32], BF16, name="selsb")
            nc.sync.dma_start(out=selsb, in_=sel_d)
            ursb = cpool.tile([O, T], F32, name="ursb")
            nc.sync.dma_start(out=ursb, in_=ur_d)
            brusb = cpool.tile([O, 1], F32, name="brusb")
            nc.sync.dma_start(out=brusb, in_=bru_d)

            mhat = spool.tile([128, 2 * NBB * NSL], BF16, name="mhat")
            sfull = spool.tile([128, 2 * NBB * NSL], BF16, name="sfull")
            q = mpool.tile([128, 64], BF16, name="q")
            cnt = mpool.tile([128, 4], F32, name="cnt")
            csum = mpool.tile([128, 2], F32, name="csum")
            par = mpool.tile([128, 2], F32, name="par")
            acc = mpool.tile([O, BL], F32, name="acc")
            accb = mpool.tile([O, BL], F32, name="accb")
            zjunk = mpool.tile([O, T], F32, name="zjunk")

            mh_v = mhat.rearrange("p (hh b t) -> p hh b t", hh=2, b=BL, t=T)
            sf_v = sfull.rearrange("p (hh b t) -> p hh b t", hh=2, b=BL, t=T)
            q_v = q.rearrange("p (hh b) -> p hh b", hh=2)

            with tc.tile_pool(name="psA", bufs=2, space="PSUM") as pspool:

                def layer(L, bsl_d, rhs_mm):
                    bslsb = bpool.tile([128, NCF * NSL], BF16, name=f"bslsb{L}",
                                       tag="bsl")
                    for cf in range(NCF):
                        nc.sync.dma_start(out=bslsb[:, cf * NSL:(cf + 1) * NSL],
                                          in_=bsl_d[cf])
                    aoff = (L - 1) * 2 * NSL
                    for bb in range(NBB):
                        ds = dpool.tile([128, NCF * NSL], BF16,
                                        name=f"ds{L}_{bb}", tag="ds")
                        for cf in range(NCF):
                            ps = pspool.tile([128, NSL], F32,
                                             name=f"c{L}_{bb}_{cf}", tag="mm")
                            for nn in range(2):
                                rhs_mm(ps, bb, cf, nn)
                            nc.vector.tensor_tensor_scan(
                                out=ds[:, cf * NSL:(cf + 1) * NSL],
                                data0=bslsb[:, cf * NSL:(cf + 1) * NSL],
                                data1=ps,
                                initial=0.0, op0=ALU.mult, op1=ALU.add)
                        for hh in range(2):
                            Dps = pspool.tile([128, 1024], F32,
                                              name=f"D{L}_{bb}_{hh}", tag="D")
                            for c4 in range(4):
                                o4 = (hh * 4 + c4) * NSL
                                for n0, nw in NN_SPLITS:
                                    nc.tensor.matmul(
                                        Dps[c4 * 32:(c4 + 1) * 32,
                                            n0:n0 + nw],
                                        lhsT=selsb,
                                        rhs=ds[:, o4 + n0:o4 + n0 + nw],
                                        start=True, stop=True,
                                        tile_position=(0, c4 * 32))
                            nc.vector.tensor_tensor_scan(
                                out=mhat[:, hh * 8000 + bb * NSL:
                                         hh * 8000 + (bb + 1) * NSL],
                                data0=aslsb[:, aoff + hh * NSL:
                                            aoff + (hh + 1) * NSL],
                                data1=Dps[:, 0:NSL], initial=0.0,
                                op0=ALU.mult, op1=ALU.add)

                def spike_phase(L):
                    nc.gpsimd.memset(sfull, 0.0)
                    junk = dpool.tile([128, NCF * NSL], BF16,
                                      name=f"junk{L}", tag="ds")
                    for hh in range(2):
                        nc.vector.tensor_scalar(
                            out=junk[:, 0:8000],
                            in0=mhat[:, hh * 8000:(hh + 1) * 8000],
                            scalar1=float(VTH), scalar2=None, op0=ALU.is_gt,
                            op1=ALU.add,
                            accum_out=cnt[:, (L - 1) * 2 + hh:(L - 1) * 2 + hh + 1])
                    nc.vector.tensor_add(
                        out=csum[:, L - 1:L],
                        in0=cnt[:, (L - 1) * 2:(L - 1) * 2 + 1],
                        in1=cnt[:, (L - 1) * 2 + 1:(L - 1) * 2 + 2])
                    nc.gpsimd.partition_all_reduce(
                        par[:, L - 1:L], csum[:, L - 1:L], channels=128,
                        reduce_op=bass_isa.ReduceOp.add)
                    nc.vector.memset(q, 0.0)
                    for t in range(T):
                        nc.vector.scalar_tensor_tensor(
                            out=sf_v[:, :, :, t], in0=mh_v[:, :, :, t],
                            scalar=float(VTH), op0=ALU.subtract,
                            in1=q_v, op1=ALU.is_gt)
                        for hh in range(2):
                            nc.vector.scalar_tensor_tensor(
                                out=q[:, hh * 32:(hh + 1) * 32],
                                in0=q[:, hh * 32:(hh + 1) * 32],
                                scalar=acolsb[:, (L - 1) * 2 + hh:
                                              (L - 1) * 2 + hh + 1],
                                op0=ALU.mult,
                                in1=sf_v[:, hh, :, t], op1=ALU.add)

                xs = {}

                def mm1(ps, bb, cf, nn):
                    n0, nw = NN_SPLITS[nn]
                    if cf == 0 and nn == 0:
                        for i in range(IC):
                            t_ = xpool.tile([128, NSL], BF16,
                                            name=f"xs{bb}_{i}", tag=f"xs{i}")
                            nc.sync.dma_start(
                                out=t_.rearrange("p (b t) -> p b t", b=BBLK),
                                in_=xt_d[i * 128:(i + 1) * 128,
                                         bb * BBLK:(bb + 1) * BBLK, :])
                            xs[i] = t_
                    for i in range(IC):
                        nc.tensor.matmul(
                            ps[:, n0:n0 + nw],
                            lhsT=w1sb[i][:, cf * 128:(cf + 1) * 128],
                            rhs=xs[i][:, n0:n0 + nw],
                            start=(i == 0), stop=(i == IC - 1))

                layer(1, bsl1_d, mm1)
                spike_phase(1)

                def mm2(ps, bb, cf, nn):
                    n0, nw = NN_SPLITS[nn]
                    for hh in range(2):
                        nc.tensor.matmul(
                            ps[:, n0:n0 + nw],
                            lhsT=w2sb[hh][:, cf * 128:(cf + 1) * 128],
                            rhs=sfull[:, hh * 8000 + bb * NSL + n0:
                                      hh * 8000 + bb * NSL + n0 + nw],
                            start=(hh == 0), stop=(hh == 1))

                layer(2, bsl2_d, mm2)
                nc.vector.tensor_add(
                    out=mh_v, in0=mh_v,
                    in1=m2bsb.rearrange("p (hh t) -> p hh t", hh=2)
                        .unsqueeze(2).broadcast_to((128, 2, BL, T)))
                spike_phase(2)

            with tc.tile_pool(name="psB", bufs=2, space="PSUM") as zpool:
                for bb in range(NBB):
                    for nn in range(2):
                        zps = zpool.tile([O, 500], F32, name=f"z{bb}_{nn}",
                                         tag="z")
                        for hh in range(2):
                            nc.tensor.matmul(
                                zps,
                                lhsT=wrsb[:, hh * O:(hh + 1) * O],
                                rhs=sfull[:, hh * 8000 + bb * NSL + nn * 500:
                                          hh * 8000 + bb * NSL + (nn + 1) * 500],
                                start=(hh == 0), stop=(hh == 1))
                        for b2 in range(2):
                            b = bb * BBLK + nn * 2 + b2
                            nc.vector.scalar_tensor_tensor(
                                out=zjunk, in0=zps[:, b2 * T:(b2 + 1) * T],
                                scalar=1.0, op0=ALU.mult,
                                in1=ursb, op1=ALU.mult,
                                accum_out=acc[:, b:b + 1])
                nc.vector.tensor_scalar(
                    out=accb, in0=acc, scalar1=brusb[:, 0:1], scalar2=None,
                    op0=ALU.add)
                nc.sync.dma_start(out=out_d, in_=accb)
                nc.sync.dma_start(out=flag_d, in_=par[0:1, 0:2])

    nc.compile()
    return nc


_NC_CACHE = {}


def get_nc():
    if "fast" not in _NC_CACHE:
        _NC_CACHE["fast"] = build_nc()
    return _NC_CACHE["fast"]


def get_nc_slow():
    if "slow" not in _NC_CACHE:
        _NC_CACHE["slow"] = build_nc_slow()
    return _NC_CACHE["slow"]


def prep_inputs(x, W1, b1, tau_n1, tau_m1, W2, b2, tau_n2, tau_m2,
                Wr, br, tau_mr, warmup):
    """Host-side: per-core input dicts for the fast bass kernel, plus the
    host-verified layer-2/readout constants. Returns (in_maps, fast_ok)."""
    w = int(np.asarray(warmup))
    beta1 = _sig(tau_n1).reshape(NF)          # [H,K], j = h*4+k order
    alpha1 = _sig(tau_m1)                     # [H]
    beta2 = _sig(tau_n2).reshape(NF)
    alpha2 = _sig(tau_m2)
    alphar = _sig(tau_mr)                     # [O]

    g1 = (1.0 - beta1) * np.repeat(1.0 - alpha1, K)

    # fp8 weights, prescaled by WSC; row 700 = bias, rows 701.. = 0
    w1t = np.zeros((IC * 128, NF), np.float64)
    w1t[:IN] = np.asarray(W1, np.float64).T * WSC
    w1t[IN] = np.asarray(b1, np.float64) * WSC
    w1q = np.empty((NPR, 128, 2 * NF), ml_dtypes.float8_e4m3)
    for pr in range(NPR):
        w1q[pr, :, :NF] = w1t[2 * pr * 128:(2 * pr + 1) * 128]
        w1q[pr, :, NF:] = w1t[(2 * pr + 1) * 128:(2 * pr + 2) * 128]

    # selector: g/WSC weights, [128, 32] blocks per feature chunk, packed
    selm = np.zeros((128, NCF * 32), ml_dtypes.bfloat16)
    for cf in range(NCF):
        j = cf * 128 + np.arange(128)
        selm[np.arange(128), cf * 32 + np.arange(128) // 4] = g1[j] / WSC

    def bslab(beta):
        s = np.tile(beta.reshape(NCF, 128, 1).astype(ml_dtypes.bfloat16),
                    (1, 1, NSL))
        s.reshape(NCF, 128, BBLK, T)[:, :, :, 0] = 0.0
        return s

    bsl1 = bslab(beta1)

    # host-exact layer-2 bias trajectory (valid when layer 1 has no spikes)
    b2g = np.asarray(b2, np.float64) * (1.0 - beta2)
    dtraj = np.zeros(NF)
    mtraj = np.zeros(H)
    m2max = -np.inf
    for _ in range(T):
        dtraj = beta2 * dtraj + b2g
        mtraj = alpha2 * mtraj + (1.0 - alpha2) * dtraj.reshape(H, K).sum(-1)
        m2max = max(m2max, mtraj.max())
    fast_ok = bool(m2max <= VTH - 0.05)

    # host-exact readout constant (valid when layer 2 has no spikes)
    mr = np.zeros(O)
    accr = np.zeros(O)
    for t_ in range(T):
        mr = mr * alphar + (1.0 - alphar) * np.asarray(br, np.float64)
        if t_ >= w:
            accr += mr
    outc = np.tile((accr / (T - w)).astype(np.float32)[:, None], (1, BL))

    xq_full = np.zeros((IC * 128, B, T), ml_dtypes.float8_e4m3)
    xq_full[:IN] = np.asarray(x).transpose(2, 0, 1)
    xq_full[IN] = 1.0
    # pair-interleaved: [NPR, 128, 2, B, T]
    xq_full = np.ascontiguousarray(
        xq_full.reshape(NPR, 2, 128, B, T).transpose(0, 2, 1, 3, 4))

    shared = dict(w1q=w1q, selm=selm, bsl1=bsl1, outc=outc)
    in_maps = []
    for c in range(N_CORES):
        m = dict(shared)
        m["xq"] = np.ascontiguousarray(
            xq_full[:, :, :, c * BL:(c + 1) * BL, :])
        in_maps.append(m)
    return in_maps, fast_ok


def prep_inputs_slow(x, W1, b1, tau_n1, tau_m1, W2, b2, tau_n2, tau_m2,
                     Wr, br, tau_mr, warmup):
    """Host-side prep for the general fallback kernel."""
    w = int(np.asarray(warmup))
    beta1 = _sig(tau_n1).reshape(NF)
    alpha1 = _sig(tau_m1)
    beta2 = _sig(tau_n2).reshape(NF)
    alpha2 = _sig(tau_m2)
    alphar = _sig(tau_mr)

    g1 = (1.0 - beta1) * np.repeat(1.0 - alpha1, K)
    g2 = (1.0 - beta2) * np.repeat(1.0 - alpha2, K)

    w1p = np.zeros((IC * 128, NF), np.float64)
    w1p[:IN] = np.asarray(W1, np.float64).T * g1
    w1p[IN] = np.asarray(b1, np.float64) * g1
    w1p = w1p.astype(ml_dtypes.bfloat16)

    w2p = (np.asarray(W2, np.float64).T * g2).astype(ml_dtypes.bfloat16)
    b2g = np.asarray(b2, np.float64) * g2
    dtraj = np.zeros(NF)
    mh2b = np.zeros((H, T))
    mtraj = np.zeros(H)
    for t_ in range(T):
        dtraj = _sig(tau_n2).reshape(NF) * dtraj + b2g
        mtraj = _sig(tau_m2) * mtraj + dtraj.reshape(H, K).sum(-1)
        mh2b[:, t_] = mtraj
    mh2b_dev = np.zeros((128, 2 * T), np.float64)
    mh2b_dev[:, :T] = mh2b[:128]
    mh2b_dev[:, T:] = mh2b[128:]
    mh2b_dev = mh2b_dev.astype(ml_dtypes.bfloat16)

    wrt = np.zeros((128, 2 * O), np.float64)
    wrt[:, :O] = np.asarray(Wr, np.float64).T[:128]
    wrt[:, O:] = np.asarray(Wr, np.float64).T[128:]
    wrt = wrt.astype(ml_dtypes.bfloat16)

    def bslab(beta):
        s = np.tile(beta.reshape(NCF, 128, 1).astype(ml_dtypes.bfloat16),
                    (1, 1, NSL))
        s.reshape(NCF, 128, BBLK, T)[:, :, :, 0] = 0.0
        return s

    bsl1 = bslab(beta1)
    bsl2 = bslab(beta2)

    def aslab(alpha):
        a2 = alpha.reshape(2, 128).astype(ml_dtypes.bfloat16)
        s = np.tile(a2[:, :, None], (1, 1, NSL))
        s.reshape(2, 128, BBLK, T)[:, :, :, 0] = 0.0
        return s

    asl = np.concatenate([aslab(alpha1), aslab(alpha2)], axis=0)
    asl = asl.transpose(1, 0, 2).reshape(128, 4 * NSL).copy()

    acol = np.stack([alpha1[:128], alpha1[128:], alpha2[:128], alpha2[128:]],
                    axis=1).astype(np.float32)

    selm = np.zeros((128, 32), ml_dtypes.bfloat16)
    selm[np.arange(128), np.arange(128) // 4] = 1.0

    tt = np.arange(T, dtype=np.float64)[:, None]
    ar = alphar[None, :]
    u = ar ** np.maximum(0, w - tt) - ar ** (T - tt)
    ur = (u.T / (T - w)).astype(np.float32)
    bru = (np.asarray(br, np.float64) * u.sum(0) / (T - w)) \
        .astype(np.float32)[:, None]

    xt_full = np.zeros((IC * 128, B, T), ml_dtypes.bfloat16)
    xt_full[:IN] = np.asarray(x).transpose(2, 0, 1)
    xt_full[IN] = 1.0

    shared = dict(w1p=w1p, w2p=w2p, mh2b=mh2b_dev, wrt=wrt,
                  bsl1=bsl1, bsl2=bsl2, asl=asl, acol=acol, selm=selm,
                  ur=ur, bru=bru)
    in_maps = []
    for c in range(N_CORES):
        m = dict(shared)
        m["xt"] = np.ascontiguousarray(xt_full[:, c * BL:(c + 1) * BL, :])
        in_maps.append(m)
    return in_maps


def _run_slow(**inputs):
    in_maps = prep_inputs_slow(**inputs)
    res = bass_utils.run_bass_kernel_spmd(
        get_nc_slow(), in_maps, core_ids=list(range(N_CORES)))
    out = np.empty((B, O), np.float32)
    for c in range(N_CORES):
        out[c * BL:(c + 1) * BL] = res.results[c]["out"].T
    return out


def kernel(**inputs):
    in_maps, fast_ok = prep_inputs(**inputs)
    if not fast_ok:
        return _run_slow(**inputs)
    res = bass_utils.run_bass_kernel_spmd(
        get_nc(), in_maps, core_ids=list(range(N_CORES)))
    if any(r["flag"].sum() > 0 for r in res.results):
        # certificate failed: spikes may exist, use the general kernel
        return _run_slow(**inputs)
    out = np.empty((B, O), np.float32)
    for c in range(N_CORES):
        out[c * BL:(c + 1) * BL] = res.results[c]["out"].T
    return out


# revision 30
# speedup vs baseline: 1.0724x; 1.0448x over previous
"""DH-SFNN Trainium2 kernel (8 NeuronCores, data-parallel over batch).

Model: 2 dendritic LIF layers (K=4 branches, reset-by-subtraction) + leaky
readout integrator, T=250 steps, B=256, IN=700, H=256, O=20.

Fast path (per core, B_l=32), exploiting reset-by-subtraction soundness:
spike corrections are strictly subtractive, so if the no-spike layer-1
membrane trajectory m1^ satisfies max m1^ <= VTH there are exactly zero
layer-1 spikes. Layer 2 then sees only its bias trajectory (x-independent,
verified exactly on host), and the readout is a batch-independent constant
computed on host. The device therefore only needs to certify layer 1:

    c1 = x @ (16*W1).T (+bias row)     -- fp8 DoubleRow matmuls (2x128
                                          contraction rows per instr)
    d1 = per-feature 1-pole IIR over t -- DVE tensor_tensor_scan, 4 batch
                                          streams packed per instruction with
                                          zeroed-multiplier boundary columns
    D1 = sum_k g_k d1_k               -- PE matmul with g/16-weighted selector
    check max_t D1 <= VTH - 0.25      -- Act engine relu-accumulate; since
                                          m1^ is a running convex combination
                                          of D1, max m1^ <= max(0, max D1).

If the on-device flag fires, or the host-side layer-2 bias-trajectory check
fails, rerun with the general sequential-correction kernel (slow path).
"""
import sys

sys.path.insert(0, "/opt/trn_rl_repo")

import numpy as np
import ml_dtypes

import concourse.bass as bass
import concourse.mybir as mybir
import concourse.tile as tile
from concourse import bacc, bass_utils, bass_isa

F32 = mybir.dt.float32
BF16 = mybir.dt.bfloat16
FP8 = mybir.dt.float8e4
ALU = mybir.AluOpType
ACT = mybir.ActivationFunctionType
DR = mybir.MatmulPerfMode.DoubleRow

N_CORES = 8
B, T, IN, H, O, K = 256, 250, 700, 256, 20, 4
BL = B // N_CORES            # 32 batch per core
BBLK = 4                     # batches per scan slab
NBB = BL // BBLK             # 8 slabs
NSL = BBLK * T               # 1000 slab columns
IC = 6                       # 768 = 6*128 contraction rows (row 700 = bias)
NPR = IC // 2                # 3 DoubleRow pair chunks
NF = H * K                   # 1024 layer-1 branch features
NCF = NF // 128              # 8 feature chunks
VTH = 1.0
CHECK_MARGIN = 0.25          # device certifies max D <= VTH - margin
WSC = 16.0                   # power-of-2 prescale on W1 for fp8 range
# out-column splits of the 1000 slab columns, each within one PSUM bank
CSPLITS = [(0, 256), (256, 256), (512, 256), (768, 232)]
NN_SPLITS = [(0, 512), (512, 488)]


def _sig(v):
    return 1.0 / (1.0 + np.exp(-np.asarray(v, np.float64)))


def build_nc():
    nc = bacc.Bacc("TRN2", target_bir_lowering=False, debug=False,
                   num_devices=N_CORES)
    dt = nc.dram_tensor
    xq_d = dt("xq", [NPR, 128, 2, BL, T], FP8, kind="ExternalInput").ap()
    w1_d = dt("w1q", [NPR, 128, 2 * NF], FP8, kind="ExternalInput").ap()
    sel_d = dt("selm", [128, NCF * 32], BF16, kind="ExternalInput").ap()
    bcol_d = dt("betacol", [128, NCF], F32, kind="ExternalInput").ap()
    outc_d = dt("outc", [O, BL], F32, kind="ExternalInput").ap()
    out_d = dt("out", [O, BL], F32, kind="ExternalOutput").ap()
    flag_d = dt("flag", [128, 2 * NBB], F32, kind="ExternalOutput").ap()

    with tile.TileContext(nc) as tc:
        with tc.tile_pool(name="const", bufs=1) as cpool, \
             tc.tile_pool(name="xs", bufs=2) as xpool, \
             tc.tile_pool(name="ds", bufs=2) as dpool, \
             tc.tile_pool(name="small", bufs=1) as mpool:

            # ---- constants ----
            # wire order: betacol, w1 cf0/1-columns, x slab 0, w1 rest.
            # beta slabs are built on the Act engine (mask * beta-column)
            # instead of DMA'd, keeping the serial DMA wire free for x.
            w1sb = [cpool.tile([128, 2 * NF], FP8, name=f"w1sb{i}",
                               tag=f"w1_{i}") for i in range(NPR)]
            bslsb = cpool.tile([128, NCF * NSL], BF16, name="bslsb")
            bcolsb = cpool.tile([128, NCF], F32, name="bcolsb")
            nc.sync.dma_start(out=bcolsb, in_=bcol_d)
            w1v = [w.rearrange("p (k m) -> p k m", k=2) for w in w1sb]
            for i in range(NPR):
                nc.sync.dma_start(out=w1v[i][:, :, 0:256],
                                  in_=w1_d[i].rearrange("p (k m) -> p k m",
                                                        k=2)[:, :, 0:256])
            selsb = cpool.tile([128, NCF * 32], BF16, name="selsb")
            outcsb = cpool.tile([O, BL], F32, name="outcsb")
            biasc = mpool.tile([128, 1], F32, name="biasc")
            nc.vector.memset(biasc, -(VTH - CHECK_MARGIN))
            # PE p-state warmup: dummy matmuls on a zeroed scratch tile keep
            # the tensor engine ramping while the first DMAs land.
            wscr = mpool.tile([128, 128], BF16, name="wscr")
            nc.vector.memset(wscr, 0.0)
            # mask for the beta slabs: ones, zero at each batch-stream start
            mask = mpool.tile([128, NSL], BF16, name="mask")
            nc.vector.memset(mask, 1.0)
            for b_ in range(BBLK):
                nc.vector.memset(mask[:, b_ * T:b_ * T + 1], 0.0)
            for cf in range(NCF):
                nc.scalar.activation(out=bslsb[:, cf * NSL:(cf + 1) * NSL],
                                     in_=mask, func=ACT.Copy, bias=0.0,
                                     scale=bcolsb[:, cf:cf + 1])

            cnt = mpool.tile([128, 2 * NBB], F32, name="cnt")
            csum = mpool.tile([128, 1], F32, name="csum")
            junk = mpool.tile([128, NSL], BF16, name="junk")
            junk16 = mpool.tile([128, 2 * NBB], F32, name="junk16")

            with tc.tile_pool(name="psA", bufs=2, space="PSUM") as pspool, \
                 tc.tile_pool(name="psB", bufs=2, space="PSUM") as dppool:
                dss = {}

                def emit_x(bb, spread=False):
                    xs = []
                    eng = [nc.gpsimd] * NPR
                    if spread:
                        eng = [nc.sync, nc.sync, nc.gpsimd]
                    for pr in range(NPR):
                        t_ = xpool.tile([128, 2 * NSL], FP8,
                                        name=f"xs{bb}_{pr}", tag=f"xs{pr}")
                        eng[pr].dma_start(
                            out=t_.rearrange("p (k b t) -> p k b t",
                                             k=2, b=BBLK),
                            in_=xq_d[pr][:, :, bb * BBLK:(bb + 1) * BBLK, :])
                        xs.append(t_.rearrange("p (k n) -> p k n", k=2))
                    return xs

                def emit_cmm_scan(bb, xs, cfs, pre_cf=None):
                    ds = dss[bb]
                    for cf in cfs:
                        if pre_cf is not None:
                            pre_cf(cf)
                        ps = pspool.tile([128, 1024], F32,
                                         name=f"c{bb}_{cf}", tag="mm")
                        for n0, nw in CSPLITS:
                            for pr in range(NPR):
                                nc.tensor.matmul(
                                    ps[:, n0:n0 + nw],
                                    lhsT=w1sb[pr]
                                        .rearrange("p (k m) -> p k m", k=2)
                                        [:, :, cf * 128:(cf + 1) * 128],
                                    rhs=xs[pr][:, :, n0:n0 + nw],
                                    start=(pr == 0), stop=(pr == NPR - 1),
                                    perf_mode=DR)
                        nc.vector.tensor_tensor_scan(
                            out=ds[:, cf * NSL:(cf + 1) * NSL],
                            data0=bslsb[:, cf * NSL:(cf + 1) * NSL],
                            data1=ps[:, 0:NSL], initial=0.0,
                            op0=ALU.mult, op1=ALU.add)

                Dcur = {}

                def emit_sel_mm(bb, hh, c4s):
                    ds = dss[bb]
                    if (bb, hh) not in Dcur:
                        Dcur[(bb, hh)] = dppool.tile(
                            [128, 1024], F32, name=f"D{bb}_{hh}", tag="D")
                    Dps = Dcur[(bb, hh)]
                    for c4 in c4s:
                        cf = hh * 4 + c4
                        for n0, nw in NN_SPLITS:
                            nc.tensor.matmul(
                                Dps[c4 * 32:(c4 + 1) * 32, n0:n0 + nw],
                                lhsT=selsb[:, cf * 32:(cf + 1) * 32],
                                rhs=ds[:, cf * NSL + n0:cf * NSL + n0 + nw],
                                start=True, stop=True,
                                tile_position=(0, c4 * 32))

                def emit_check(bb, hh):
                    # spike certificate: relu(D - (VTH - margin)) summed
                    Dps = Dcur.pop((bb, hh))
                    nc.scalar.activation(
                        out=junk, in_=Dps[:, 0:NSL], func=ACT.Relu,
                        bias=biasc, scale=1.0,
                        accum_out=cnt[:, bb * 2 + hh:bb * 2 + hh + 1])

                def emit_sel_check(bb, hh):
                    emit_sel_mm(bb, hh, range(4))
                    emit_check(bb, hh)

                for bb in range(NBB):
                    dss[bb] = dpool.tile([128, NCF * NSL], BF16,
                                         name=f"ds{bb}", tag="ds")
                xs = emit_x(0, spread=True)
                # rest of the weight columns (cf2..7) after slab-0 x
                for i in range(NPR):
                    nc.sync.dma_start(out=w1v[i][:, :, 256:NF],
                                      in_=w1_d[i].rearrange(
                                          "p (k m) -> p k m",
                                          k=2)[:, :, 256:NF])
                nc.sync.dma_start(out=selsb, in_=sel_d)
                nc.sync.dma_start(out=outcsb, in_=outc_d)
                nc.sync.dma_start(out=out_d, in_=outcsb)
                # warmup: ~3us of dummy PE work to reach full clock before
                # the first real matmul
                wps = pspool.tile([128, 1024], F32, name="warm", tag="mm")
                for i in range(30):
                    nc.tensor.matmul(wps[0:128, 0:128], lhsT=wscr, rhs=wscr,
                                     start=True, stop=True)

                last = NBB - 1
                for bb in range(NBB):
                    emit_cmm_scan(bb, xs, range(0, 4))
                    if bb > 0:
                        emit_sel_check(bb - 1, 0)
                    if bb < last:
                        emit_cmm_scan(bb, xs, range(4, NCF))
                        xs = emit_x(bb + 1)
                        if bb > 0:
                            emit_sel_check(bb - 1, 1)
                    else:
                        # final slab: emit matmuls ahead so DVE stays fed,
                        # then chase the last scans with small selector slices
                        emit_cmm_scan(bb, xs, range(4, NCF))
                        emit_sel_check(bb - 1, 1)
                        emit_sel_mm(bb, 0, range(4))
                        emit_check(bb, 0)
                        for c4 in range(4):
                            emit_sel_mm(bb, 1, [c4])
                        emit_check(bb, 1)

            nc.sync.dma_start(out=flag_d, in_=cnt)

    nc.compile()
    return nc


# ---------------------------------------------------------------------------
# general fallback kernel (sequential spike-correction), used only when the
# no-spike certificate fails: identical to the reference recurrence.
# ---------------------------------------------------------------------------

def build_nc_slow():
    nc = bacc.Bacc("TRN2", target_bir_lowering=False, debug=False,
                   num_devices=N_CORES)
    dt = nc.dram_tensor
    xt_d = dt("xt", [IC * 128, BL, T], BF16, kind="ExternalInput").ap()
    w1_d = dt("w1p", [IC * 128, NF], BF16, kind="ExternalInput").ap()
    w2_d = dt("w2p", [H, NF], BF16, kind="ExternalInput").ap()
    wr_d = dt("wrt", [128, 2 * O], BF16, kind="ExternalInput").ap()
    m2b_d = dt("mh2b", [128, 2 * T], BF16, kind="ExternalInput").ap()
    bsl1_d = dt("bsl1", [NCF, 128, NSL], BF16, kind="ExternalInput").ap()
    bsl2_d = dt("bsl2", [NCF, 128, NSL], BF16, kind="ExternalInput").ap()
    asl_d = dt("asl", [128, 4 * NSL], BF16, kind="ExternalInput").ap()
    acol_d = dt("acol", [128, 4], F32, kind="ExternalInput").ap()
    sel_d = dt("selm", [128, 32], BF16, kind="ExternalInput").ap()
    ur_d = dt("ur", [O, T], F32, kind="ExternalInput").ap()
    bru_d = dt("bru", [O, 1], F32, kind="ExternalInput").ap()
    out_d = dt("out", [O, BL], F32, kind="ExternalOutput").ap()
    flag_d = dt("flag", [1, 2], F32, kind="ExternalOutput").ap()

    with tile.TileContext(nc) as tc:
        with tc.tile_pool(name="const", bufs=1) as cpool, \
             tc.tile_pool(name="state", bufs=1) as spool, \
             tc.tile_pool(name="bsl", bufs=1) as bpool, \
             tc.tile_pool(name="xs", bufs=2) as xpool, \
             tc.tile_pool(name="ds", bufs=2) as dpool, \
             tc.tile_pool(name="small", bufs=1) as mpool:

            w1sb = [cpool.tile([128, NF], BF16, name=f"w1sb{i}", tag=f"w1_{i}")
                    for i in range(IC)]
            for i in range(IC):
                nc.sync.dma_start(out=w1sb[i], in_=w1_d[i * 128:(i + 1) * 128, :])
            w2sb = [cpool.tile([128, NF], BF16, name=f"w2sb{i}", tag=f"w2_{i}")
                    for i in range(2)]
            for i in range(2):
                nc.sync.dma_start(out=w2sb[i], in_=w2_d[i * 128:(i + 1) * 128, :])
            wrsb = cpool.tile([128, 2 * O], BF16, name="wrsb")
            nc.sync.dma_start(out=wrsb, in_=wr_d)
            m2bsb = cpool.tile([128, 2 * T], BF16, name="m2bsb")
            nc.sync.dma_start(out=m2bsb, in_=m2b_d)
            aslsb = cpool.tile([128, 4 * NSL], BF16, name="aslsb")
            nc.sync.dma_start(out=aslsb, in_=asl_d)
            acolsb = cpool.tile([128, 4], F32, name="acolsb")
            nc.sync.dma_start(out=acolsb, in_=acol_d)
            selsb = cpool.tile([128, 32], BF16, name="selsb")
            nc.sync.dma_start(out=selsb, in_=sel_d)
            ursb = cpool.tile([O, T], F32, name="ursb")
            nc.sync.dma_start(out=ursb, in_=ur_d)
            brusb = cpool.tile([O, 1], F32, name="brusb")
            nc.sync.dma_start(out=brusb, in_=bru_d)

            mhat = spool.tile([128, 2 * NBB * NSL], BF16, name="mhat")
            sfull = spool.tile([128, 2 * NBB * NSL], BF16, name="sfull")
            q = mpool.tile([128, 64], BF16, name="q")
            cnt = mpool.tile([128, 4], F32, name="cnt")
            csum = mpool.tile([128, 2], F32, name="csum")
            par = mpool.tile([128, 2], F32, name="par")
            acc = mpool.tile([O, BL], F32, name="acc")
            accb = mpool.tile([O, BL], F32, name="accb")
            zjunk = mpool.tile([O, T], F32, name="zjunk")

            mh_v = mhat.rearrange("p (hh b t) -> p hh b t", hh=2, b=BL, t=T)
            sf_v = sfull.rearrange("p (hh b t) -> p hh b t", hh=2, b=BL, t=T)
            q_v = q.rearrange("p (hh b) -> p hh b", hh=2)

            with tc.tile_pool(name="psA", bufs=2, space="PSUM") as pspool:

                def layer(L, bsl_d, rhs_mm):
                    bslsb = bpool.tile([128, NCF * NSL], BF16, name=f"bslsb{L}",
                                       tag="bsl")
                    for cf in range(NCF):
                        nc.sync.dma_start(out=bslsb[:, cf * NSL:(cf + 1) * NSL],
                                          in_=bsl_d[cf])
                    aoff = (L - 1) * 2 * NSL
                    for bb in range(NBB):
                        ds = dpool.tile([128, NCF * NSL], BF16,
                                        name=f"ds{L}_{bb}", tag="ds")
                        for cf in range(NCF):
                            ps = pspool.tile([128, NSL], F32,
                                             name=f"c{L}_{bb}_{cf}", tag="mm")
                            for nn in range(2):
                                rhs_mm(ps, bb, cf, nn)
                            nc.vector.tensor_tensor_scan(
                                out=ds[:, cf * NSL:(cf + 1) * NSL],
                                data0=bslsb[:, cf * NSL:(cf + 1) * NSL],
                                data1=ps,
                                initial=0.0, op0=ALU.mult, op1=ALU.add)
                        for hh in range(2):
                            Dps = pspool.tile([128, 1024], F32,
                                              name=f"D{L}_{bb}_{hh}", tag="D")
                            for c4 in range(4):
                                o4 = (hh * 4 + c4) * NSL
                                for n0, nw in NN_SPLITS:
                                    nc.tensor.matmul(
                                        Dps[c4 * 32:(c4 + 1) * 32,
                                            n0:n0 + nw],
                                        lhsT=selsb,
                                        rhs=ds[:, o4 + n0:o4 + n0 + nw],
                                        start=True, stop=True,
                                        tile_position=(0, c4 * 32))
                            nc.vector.tensor_tensor_scan(
                                out=mhat[:, hh * 8000 + bb * NSL:
                                         hh * 8000 + (bb + 1) * NSL],
                                data0=aslsb[:, aoff + hh * NSL:
                                            aoff + (hh + 1) * NSL],
                                data1=Dps[:, 0:NSL], initial=0.0,
                                op0=ALU.mult, op1=ALU.add)

                def spike_phase(L):
                    nc.gpsimd.memset(sfull, 0.0)
                    junk = dpool.tile([128, NCF * NSL], BF16,
                                      name=f"junk{L}", tag="ds")
                    for hh in range(2):
                        nc.vector.tensor_scalar(
                            out=junk[:, 0:8000],
                            in0=mhat[:, hh * 8000:(hh + 1) * 8000],
                            scalar1=float(VTH), scalar2=None, op0=ALU.is_gt,
                            op1=ALU.add,
                            accum_out=cnt[:, (L - 1) * 2 + hh:(L - 1) * 2 + hh + 1])
                    nc.vector.tensor_add(
                        out=csum[:, L - 1:L],
                        in0=cnt[:, (L - 1) * 2:(L - 1) * 2 + 1],
                        in1=cnt[:, (L - 1) * 2 + 1:(L - 1) * 2 + 2])
                    nc.gpsimd.partition_all_reduce(
                        par[:, L - 1:L], csum[:, L - 1:L], channels=128,
                        reduce_op=bass_isa.ReduceOp.add)
                    nc.vector.memset(q, 0.0)
                    for t in range(T):
                        nc.vector.scalar_tensor_tensor(
                            out=sf_v[:, :, :, t], in0=mh_v[:, :, :, t],
                            scalar=float(VTH), op0=ALU.subtract,
                            in1=q_v, op1=ALU.is_gt)
                        for hh in range(2):
                            nc.vector.scalar_tensor_tensor(
                                out=q[:, hh * 32:(hh + 1) * 32],
                                in0=q[:, hh * 32:(hh + 1) * 32],
                                scalar=acolsb[:, (L - 1) * 2 + hh:
                                              (L - 1) * 2 + hh + 1],
                                op0=ALU.mult,
                                in1=sf_v[:, hh, :, t], op1=ALU.add)

                xs = {}

                def mm1(ps, bb, cf, nn):
                    n0, nw = NN_SPLITS[nn]
                    if cf == 0 and nn == 0:
                        for i in range(IC):
                            t_ = xpool.tile([128, NSL], BF16,
                                            name=f"xs{bb}_{i}", tag=f"xs{i}")
                            nc.sync.dma_start(
                                out=t_.rearrange("p (b t) -> p b t", b=BBLK),
                                in_=xt_d[i * 128:(i + 1) * 128,
                                         bb * BBLK:(bb + 1) * BBLK, :])
                            xs[i] = t_
                    for i in range(IC):
                        nc.tensor.matmul(
                            ps[:, n0:n0 + nw],
                            lhsT=w1sb[i][:, cf * 128:(cf + 1) * 128],
                            rhs=xs[i][:, n0:n0 + nw],
                            start=(i == 0), stop=(i == IC - 1))

                layer(1, bsl1_d, mm1)
                spike_phase(1)

                def mm2(ps, bb, cf, nn):
                    n0, nw = NN_SPLITS[nn]
                    for hh in range(2):
                        nc.tensor.matmul(
                            ps[:, n0:n0 + nw],
                            lhsT=w2sb[hh][:, cf * 128:(cf + 1) * 128],
                            rhs=sfull[:, hh * 8000 + bb * NSL + n0:
                                      hh * 8000 + bb * NSL + n0 + nw],
                            start=(hh == 0), stop=(hh == 1))

                layer(2, bsl2_d, mm2)
                nc.vector.tensor_add(
                    out=mh_v, in0=mh_v,
                    in1=m2bsb.rearrange("p (hh t) -> p hh t", hh=2)
                        .unsqueeze(2).broadcast_to((128, 2, BL, T)))
                spike_phase(2)

            with tc.tile_pool(name="psB", bufs=2, space="PSUM") as zpool:
                for bb in range(NBB):
                    for nn in range(2):
                        zps = zpool.tile([O, 500], F32, name=f"z{bb}_{nn}",
                                         tag="z")
                        for hh in range(2):
                            nc.tensor.matmul(
                                zps,
                                lhsT=wrsb[:, hh * O:(hh + 1) * O],
                                rhs=sfull[:, hh * 8000 + bb * NSL + nn * 500:
                                          hh * 8000 + bb * NSL + (nn + 1) * 500],
                                start=(hh == 0), stop=(hh == 1))
                        for b2 in range(2):
                            b = bb * BBLK + nn * 2 + b2
                            nc.vector.scalar_tensor_tensor(
                                out=zjunk, in0=zps[:, b2 * T:(b2 + 1) * T],
                                scalar=1.0, op0=ALU.mult,
                                in1=ursb, op1=ALU.mult,
                                accum_out=acc[:, b:b + 1])
                nc.vector.tensor_scalar(
                    out=accb, in0=acc, scalar1=brusb[:, 0:1], scalar2=None,
                    op0=ALU.add)
                nc.sync.dma_start(out=out_d, in_=accb)
                nc.sync.dma_start(out=flag_d, in_=par[0:1, 0:2])

    nc.compile()
    return nc


_NC_CACHE = {}


def get_nc():
    if "fast" not in _NC_CACHE:
        _NC_CACHE["fast"] = build_nc()
    return _NC_CACHE["fast"]


def get_nc_slow():
    if "slow" not in _NC_CACHE:
        _NC_CACHE["slow"] = build_nc_slow()
    return _NC_CACHE["slow"]


def prep_inputs(x, W1, b1, tau_n1, tau_m1, W2, b2, tau_n2, tau_m2,
                Wr, br, tau_mr, warmup):
    """Host-side: per-core input dicts for the fast bass kernel, plus the
    host-verified layer-2/readout constants. Returns (in_maps, fast_ok)."""
    w = int(np.asarray(warmup))
    beta1 = _sig(tau_n1).reshape(NF)          # [H,K], j = h*4+k order
    alpha1 = _sig(tau_m1)                     # [H]
    beta2 = _sig(tau_n2).reshape(NF)
    alpha2 = _sig(tau_m2)
    alphar = _sig(tau_mr)                     # [O]

    g1 = (1.0 - beta1) * np.repeat(1.0 - alpha1, K)

    # fp8 weights, prescaled by WSC; row 700 = bias, rows 701.. = 0
    w1t = np.zeros((IC * 128, NF), np.float64)
    w1t[:IN] = np.asarray(W1, np.float64).T * WSC
    w1t[IN] = np.asarray(b1, np.float64) * WSC
    w1q = np.empty((NPR, 128, 2 * NF), ml_dtypes.float8_e4m3)
    for pr in range(NPR):
        w1q[pr, :, :NF] = w1t[2 * pr * 128:(2 * pr + 1) * 128]
        w1q[pr, :, NF:] = w1t[(2 * pr + 1) * 128:(2 * pr + 2) * 128]

    # selector: g/WSC weights, [128, 32] blocks per feature chunk, packed
    selm = np.zeros((128, NCF * 32), ml_dtypes.bfloat16)
    for cf in range(NCF):
        j = cf * 128 + np.arange(128)
        selm[np.arange(128), cf * 32 + np.arange(128) // 4] = g1[j] / WSC

    betacol = np.ascontiguousarray(
        beta1.reshape(NCF, 128).T.astype(np.float32))

    # host-exact layer-2 bias trajectory (valid when layer 1 has no spikes)
    b2g = np.asarray(b2, np.float64) * (1.0 - beta2)
    dtraj = np.zeros(NF)
    mtraj = np.zeros(H)
    m2max = -np.inf
    for _ in range(T):
        dtraj = beta2 * dtraj + b2g
        mtraj = alpha2 * mtraj + (1.0 - alpha2) * dtraj.reshape(H, K).sum(-1)
        m2max = max(m2max, mtraj.max())
    fast_ok = bool(m2max <= VTH - 0.05)

    # host-exact readout constant (valid when layer 2 has no spikes)
    mr = np.zeros(O)
    accr = np.zeros(O)
    for t_ in range(T):
        mr = mr * alphar + (1.0 - alphar) * np.asarray(br, np.float64)
        if t_ >= w:
            accr += mr
    outc = np.tile((accr / (T - w)).astype(np.float32)[:, None], (1, BL))

    xq_full = np.zeros((IC * 128, B, T), ml_dtypes.float8_e4m3)
    xq_full[:IN] = np.asarray(x).transpose(2, 0, 1)
    xq_full[IN] = 1.0
    # pair-interleaved: [NPR, 128, 2, B, T]
    xq_full = np.ascontiguousarray(
        xq_full.reshape(NPR, 2, 128, B, T).transpose(0, 2, 1, 3, 4))

    shared = dict(w1q=w1q, selm=selm, betacol=betacol, outc=outc)
    in_maps = []
    for c in range(N_CORES):
        m = dict(shared)
        m["xq"] = np.ascontiguousarray(
            xq_full[:, :, :, c * BL:(c + 1) * BL, :])
        in_maps.append(m)
    return in_maps, fast_ok


def prep_inputs_slow(x, W1, b1, tau_n1, tau_m1, W2, b2, tau_n2, tau_m2,
                     Wr, br, tau_mr, warmup):
    """Host-side prep for the general fallback kernel."""
    w = int(np.asarray(warmup))
    beta1 = _sig(tau_n1).reshape(NF)
    alpha1 = _sig(tau_m1)
    beta2 = _sig(tau_n2).reshape(NF)
    alpha2 = _sig(tau_m2)
    alphar = _sig(tau_mr)

    g1 = (1.0 - beta1) * np.repeat(1.0 - alpha1, K)
    g2 = (1.0 - beta2) * np.repeat(1.0 - alpha2, K)

    w1p = np.zeros((IC * 128, NF), np.float64)
    w1p[:IN] = np.asarray(W1, np.float64).T * g1
    w1p[IN] = np.asarray(b1, np.float64) * g1
    w1p = w1p.astype(ml_dtypes.bfloat16)

    w2p = (np.asarray(W2, np.float64).T * g2).astype(ml_dtypes.bfloat16)
    b2g = np.asarray(b2, np.float64) * g2
    dtraj = np.zeros(NF)
    mh2b = np.zeros((H, T))
    mtraj = np.zeros(H)
    for t_ in range(T):
        dtraj = _sig(tau_n2).reshape(NF) * dtraj + b2g
        mtraj = _sig(tau_m2) * mtraj + dtraj.reshape(H, K).sum(-1)
        mh2b[:, t_] = mtraj
    mh2b_dev = np.zeros((128, 2 * T), np.float64)
    mh2b_dev[:, :T] = mh2b[:128]
    mh2b_dev[:, T:] = mh2b[128:]
    mh2b_dev = mh2b_dev.astype(ml_dtypes.bfloat16)

    wrt = np.zeros((128, 2 * O), np.float64)
    wrt[:, :O] = np.asarray(Wr, np.float64).T[:128]
    wrt[:, O:] = np.asarray(Wr, np.float64).T[128:]
    wrt = wrt.astype(ml_dtypes.bfloat16)

    def bslab(beta):
        s = np.tile(beta.reshape(NCF, 128, 1).astype(ml_dtypes.bfloat16),
                    (1, 1, NSL))
        s.reshape(NCF, 128, BBLK, T)[:, :, :, 0] = 0.0
        return s

    bsl1 = bslab(beta1)
    bsl2 = bslab(beta2)

    def aslab(alpha):
        a2 = alpha.reshape(2, 128).astype(ml_dtypes.bfloat16)
        s = np.tile(a2[:, :, None], (1, 1, NSL))
        s.reshape(2, 128, BBLK, T)[:, :, :, 0] = 0.0
        return s

    asl = np.concatenate([aslab(alpha1), aslab(alpha2)], axis=0)
    asl = asl.transpose(1, 0, 2).reshape(128, 4 * NSL).copy()

    acol = np.stack([alpha1[:128], alpha1[128:], alpha2[:128], alpha2[128:]],
                    axis=1).astype(np.float32)

    selm = np.zeros((128, 32), ml_dtypes.bfloat16)
    selm[np.arange(128), np.arange(128) // 4] = 1.0

    tt = np.arange(T, dtype=np.float64)[:, None]
    ar = alphar[None, :]
    u = ar ** np.maximum(0, w - tt) - ar ** (T - tt)
    ur = (u.T / (T - w)).astype(np.float32)
    bru = (np.asarray(br, np.float64) * u.sum(0) / (T - w)) \
        .astype(np.float32)[:, None]

    xt_full = np.zeros((IC * 128, B, T), ml_dtypes.bfloat16)
    xt_full[:IN] = np.asarray(x).transpose(2, 0, 1)
    xt_full[IN] = 1.0

    shared = dict(w1p=w1p, w2p=w2p, mh2b=mh2b_dev, wrt=wrt,
                  bsl1=bsl1, bsl2=bsl2, asl=asl, acol=acol, selm=selm,
                  ur=ur, bru=bru)
    in_maps = []
    for c in range(N_CORES):
        m = dict(shared)
        m["xt"] = np.ascontiguousarray(xt_full[:, c * BL:(c + 1) * BL, :])
        in_maps.append(m)
    return in_maps


def _run_slow(**inputs):
    in_maps = prep_inputs_slow(**inputs)
    res = bass_utils.run_bass_kernel_spmd(
        get_nc_slow(), in_maps, core_ids=list(range(N_CORES)))
    out = np.empty((B, O), np.float32)
    for c in range(N_CORES):
        out[c * BL:(c + 1) * BL] = res.results[c]["out"].T
    return out


def kernel(**inputs):
    in_maps, fast_ok = prep_inputs(**inputs)
    if not fast_ok:
        return _run_slow(**inputs)
    res = bass_utils.run_bass_kernel_spmd(
        get_nc(), in_maps, core_ids=list(range(N_CORES)))
    if any(r["flag"].sum() > 0 for r in res.results):
        # certificate failed: spikes may exist, use the general kernel
        return _run_slow(**inputs)
    out = np.empty((B, O), np.float32)
    for c in range(N_CORES):
        out[c * BL:(c + 1) * BL] = res.results[c]["out"].T
    return out


# revision 36
# speedup vs baseline: 1.0831x; 1.0100x over previous
"""DH-SFNN Trainium2 kernel (8 NeuronCores, data-parallel over batch).

Model: 2 dendritic LIF layers (K=4 branches, reset-by-subtraction) + leaky
readout integrator, T=250 steps, B=256, IN=700, H=256, O=20.

Fast path (per core, B_l=32), exploiting reset-by-subtraction soundness:
spike corrections are strictly subtractive, so if the no-spike layer-1
membrane trajectory m1^ satisfies max m1^ <= VTH there are exactly zero
layer-1 spikes. Layer 2 then sees only its bias trajectory (x-independent,
verified exactly on host), and the readout is a batch-independent constant
computed on host. The device therefore only needs to certify layer 1:

    c1 = x @ (16*W1).T (+bias row)     -- fp8 DoubleRow matmuls (2x128
                                          contraction rows per instr)
    d1 = per-feature 1-pole IIR over t -- DVE tensor_tensor_scan, 4 batch
                                          streams packed per instruction with
                                          zeroed-multiplier boundary columns
    D1 = sum_k g_k d1_k               -- PE matmul with g/16-weighted selector
    check max_t D1 <= VTH - 0.25      -- Act engine relu-accumulate; since
                                          m1^ is a running convex combination
                                          of D1, max m1^ <= max(0, max D1).

If the on-device flag fires, or the host-side layer-2 bias-trajectory check
fails, rerun with the general sequential-correction kernel (slow path).
"""
import sys

sys.path.insert(0, "/opt/trn_rl_repo")

import numpy as np
import ml_dtypes

import concourse.bass as bass
import concourse.mybir as mybir
import concourse.tile as tile
from concourse import bacc, bass_utils, bass_isa

F32 = mybir.dt.float32
BF16 = mybir.dt.bfloat16
FP8 = mybir.dt.float8e4
ALU = mybir.AluOpType
ACT = mybir.ActivationFunctionType
DR = mybir.MatmulPerfMode.DoubleRow

N_CORES = 8
B, T, IN, H, O, K = 256, 250, 700, 256, 20, 4
BL = B // N_CORES            # 32 batch per core
BBLK = 4                     # batches per scan slab
NBB = BL // BBLK             # 8 slabs
NSL = BBLK * T               # 1000 slab columns
IC = 6                       # 768 = 6*128 contraction rows (row 700 = bias)
NPR = IC // 2                # 3 DoubleRow pair chunks
NF = H * K                   # 1024 layer-1 branch features
NCF = NF // 128              # 8 feature chunks
VTH = 1.0
CHECK_MARGIN = 0.25          # device certifies max D <= VTH - margin
WSC = 16.0                   # power-of-2 prescale on W1 for fp8 range
RBLK = 10                    # certificate block length (T = 25 blocks)
NBLK = T // RBLK
NCC = 4 * BBLK * NBLK        # certificate cols per slab: 4 chunks x 4b x 25m
# out-column splits of the 1000 slab columns, each within one PSUM bank
CSPLITS = [(0, 256), (256, 256), (512, 256), (768, 232)]
NN_SPLITS = [(0, 512), (512, 488)]


def _sig(v):
    return 1.0 / (1.0 + np.exp(-np.asarray(v, np.float64)))


def build_nc():
    nc = bacc.Bacc("TRN2", target_bir_lowering=False, debug=False,
                   num_devices=N_CORES)
    dt = nc.dram_tensor
    xq_d = dt("xq", [NPR, 128, 2, BL, T], FP8, kind="ExternalInput").ap()
    w1_d = dt("w1q", [NPR, 128, 2 * NF], FP8, kind="ExternalInput").ap()
    sel_d = dt("selm", [128, NCF * 32], BF16, kind="ExternalInput").ap()
    bcol_d = dt("betacol", [128, NCF // 2], F32, kind="ExternalInput").ap()
    brsl_d = dt("brsl", [128, 4 * BBLK * NBLK], BF16,
                kind="ExternalInput").ap()
    outc_d = dt("outc", [O, BL], F32, kind="ExternalInput").ap()
    out_d = dt("out", [O, BL], F32, kind="ExternalOutput").ap()
    flag_d = dt("flag", [128, 2 * NBB], F32, kind="ExternalOutput").ap()

    with tile.TileContext(nc) as tc:
        with tc.tile_pool(name="const", bufs=1) as cpool, \
             tc.tile_pool(name="xs", bufs=2) as xpool, \
             tc.tile_pool(name="ds", bufs=2) as dpool, \
             tc.tile_pool(name="small", bufs=1) as mpool:

            # ---- constants ----
            # wire order: betacol, w1 cf0/1-columns, x slab 0, w1 rest.
            # beta slabs (exact half, cf0-3) are built on the Act engine
            # (mask * beta-column) keeping the serial DMA wire free for x.
            w1sb = [cpool.tile([128, 2 * NF], FP8, name=f"w1sb{i}",
                               tag=f"w1_{i}") for i in range(NPR)]
            bslsb = cpool.tile([128, 4 * NSL], BF16, name="bslsb")
            bcolsb = cpool.tile([128, NCF // 2], F32, name="bcolsb")
            brslsb = cpool.tile([128, NCC], BF16, name="brslsb")
            nc.sync.dma_start(out=bcolsb, in_=bcol_d)
            w1v = [w.rearrange("p (k m) -> p k m", k=2) for w in w1sb]
            for i in range(NPR):
                nc.sync.dma_start(out=w1v[i][:, :, 0:256],
                                  in_=w1_d[i].rearrange("p (k m) -> p k m",
                                                        k=2)[:, :, 0:256])
            selsb = cpool.tile([128, NCF * 32], BF16, name="selsb")
            outcsb = cpool.tile([O, BL], F32, name="outcsb")
            biasc = mpool.tile([128, 1], F32, name="biasc")
            nc.vector.memset(biasc, -(VTH - CHECK_MARGIN))
            # certificate checks sum_k g*u against (VTH - margin)/2 since
            # max_t D in block m <= u[m-1] + P[m] <= 2 max_m u[m]
            bias2 = mpool.tile([128, 1], F32, name="bias2")
            nc.vector.memset(bias2, -(VTH - CHECK_MARGIN) / 2)
            biasz = mpool.tile([128, 1], F32, name="biasz")
            nc.vector.memset(biasz, 0.0)
            # PE p-state warmup: dummy matmuls on a zeroed scratch tile keep
            # the tensor engine ramping while the first DMAs land.
            wscr = mpool.tile([128, 128], BF16, name="wscr")
            nc.vector.memset(wscr, 0.0)
            # mask for the beta slabs: ones, zero at each batch-stream start
            mask = mpool.tile([128, NSL], BF16, name="mask")
            nc.vector.memset(mask, 1.0)
            for b_ in range(BBLK):
                nc.vector.memset(mask[:, b_ * T:b_ * T + 1], 0.0)
            for cf in range(4):
                nc.scalar.activation(out=bslsb[:, cf * NSL:(cf + 1) * NSL],
                                     in_=mask, func=ACT.Copy, bias=0.0,
                                     scale=bcolsb[:, cf:cf + 1])

            cnt = mpool.tile([128, 2 * NBB], F32, name="cnt")
            junk = mpool.tile([128, NSL], BF16, name="junk")

            with tc.tile_pool(name="psA", bufs=2, space="PSUM") as pspool, \
                 tc.tile_pool(name="psB", bufs=2, space="PSUM") as dppool, \
                 tc.tile_pool(name="rc", bufs=2) as rcpool, \
                 tc.tile_pool(name="ct", bufs=2) as ctpool:
                dss = {}
                rcs = {}
                Pts = {}

                def emit_x(bb, spread=False):
                    xs = []
                    eng = [nc.sync] * NPR
                    if spread:
                        eng = [nc.sync, nc.sync, nc.gpsimd]
                    for pr in range(NPR):
                        t_ = xpool.tile([128, 2 * NSL], FP8,
                                        name=f"xs{bb}_{pr}", tag=f"xs{pr}")
                        eng[pr].dma_start(
                            out=t_.rearrange("p (k b t) -> p k b t",
                                             k=2, b=BBLK),
                            in_=xq_d[pr][:, :, bb * BBLK:(bb + 1) * BBLK, :])
                        xs.append(t_.rearrange("p (k n) -> p k n", k=2))
                    return xs

                def emit_cmm(bb, xs, cf):
                    ps = pspool.tile([128, 1024], F32,
                                     name=f"c{bb}_{cf}", tag="mm")
                    for n0, nw in CSPLITS:
                        for pr in range(NPR):
                            nc.tensor.matmul(
                                ps[:, n0:n0 + nw],
                                lhsT=w1sb[pr]
                                    .rearrange("p (k m) -> p k m", k=2)
                                    [:, :, cf * 128:(cf + 1) * 128],
                                rhs=xs[pr][:, :, n0:n0 + nw],
                                start=(pr == 0), stop=(pr == NPR - 1),
                                perf_mode=DR)
                    return ps

                def emit_exact(bb, xs, cfs):
                    # exact path (cf 0-3): beta-IIR scan on DVE
                    ds = dss[bb]
                    for cf in cfs:
                        ps = emit_cmm(bb, xs, cf)
                        nc.vector.tensor_tensor_scan(
                            out=ds[:, cf * NSL:(cf + 1) * NSL],
                            data0=bslsb[:, cf * NSL:(cf + 1) * NSL],
                            data1=ps[:, 0:NSL], initial=0.0,
                            op0=ALU.mult, op1=ALU.add)

                def emit_cert_chunk(bb, xs, cf, tree_eng):
                    # certificate path (cf 4-7): relu on Act, block tree-sum
                    cc = cf - 4
                    rc = rcs[bb]
                    nc.scalar.activation(
                        out=rc[:, cc * NSL:(cc + 1) * NSL],
                        in_=emit_cmm(bb, xs, cf)[:, 0:NSL],
                        func=ACT.Relu, bias=biasz, scale=1.0)
                    scr = rcs[bb + 100]
                    sv = scr[:, cc * 500:(cc + 1) * 500] \
                        .rearrange("p (s r) -> p s r", s=100)
                    rv = rc[:, cc * NSL:(cc + 1) * NSL] \
                        .rearrange("p (s r) -> p s r", s=100)
                    Pv = Pts[bb][:, cc * 100:(cc + 1) * 100]
                    E = tree_eng
                    E.tensor_tensor(out=sv, in0=rv[:, :, 0:5],
                                    in1=rv[:, :, 5:10], op=ALU.add)
                    E.tensor_tensor(out=sv[:, :, 0:2], in0=sv[:, :, 0:2],
                                    in1=sv[:, :, 2:4], op=ALU.add)
                    E.tensor_tensor(out=sv[:, :, 0:1], in0=sv[:, :, 0:1],
                                    in1=sv[:, :, 1:2], op=ALU.add)
                    E.tensor_tensor(out=Pv[:, :, None], in0=sv[:, :, 0:1],
                                    in1=sv[:, :, 4:5], op=ALU.add)

                def emit_cert_tail(bb):
                    # u-scan over blocks, selector matmul on u, check
                    Pt = Pts[bb]
                    ut = ctpool.tile([128, NCC], BF16, name=f"u{bb}", tag="u")
                    nc.vector.tensor_tensor_scan(
                        out=ut, data0=brslsb, data1=Pt, initial=0.0,
                        op0=ALU.mult, op1=ALU.add)
                    Wps = dppool.tile([128, 1024], F32,
                                      name=f"W{bb}", tag="D")
                    for c4 in range(4):
                        cf = 4 + c4
                        nc.tensor.matmul(
                            Wps[c4 * 32:(c4 + 1) * 32, 0:100],
                            lhsT=selsb[:, cf * 32:(cf + 1) * 32],
                            rhs=ut[:, c4 * 100:(c4 + 1) * 100],
                            start=True, stop=True,
                            tile_position=(0, c4 * 32))
                    nc.scalar.activation(
                        out=junk[:, 0:100], in_=Wps[:, 0:100], func=ACT.Relu,
                        bias=bias2, scale=1.0,
                        accum_out=cnt[:, bb * 2 + 1:bb * 2 + 2])

                Dcur = {}

                def emit_sel_mm(bb, c4s):
                    ds = dss[bb]
                    if bb not in Dcur:
                        Dcur[bb] = dppool.tile(
                            [128, 1024], F32, name=f"D{bb}", tag="D")
                    Dps = Dcur[bb]
                    for c4 in c4s:
                        for n0, nw in NN_SPLITS:
                            nc.tensor.matmul(
                                Dps[c4 * 32:(c4 + 1) * 32, n0:n0 + nw],
                                lhsT=selsb[:, c4 * 32:(c4 + 1) * 32],
                                rhs=ds[:, c4 * NSL + n0:c4 * NSL + n0 + nw],
                                start=True, stop=True,
                                tile_position=(0, c4 * 32))

                def emit_check(bb):
                    # exact half: relu(D - (VTH - margin)) summed
                    Dps = Dcur.pop(bb)
                    nc.scalar.activation(
                        out=junk, in_=Dps[:, 0:NSL], func=ACT.Relu,
                        bias=biasc, scale=1.0,
                        accum_out=cnt[:, bb * 2:bb * 2 + 1])

                for bb in range(NBB):
                    dss[bb] = dpool.tile([128, 4 * NSL], BF16,
                                         name=f"ds{bb}", tag="ds")
                    rcs[bb] = rcpool.tile([128, 4 * NSL], BF16,
                                          name=f"rc{bb}", tag="rc")
                    rcs[bb + 100] = rcpool.tile([128, 4 * 500], BF16,
                                                name=f"scr{bb}", tag="scr")
                    Pts[bb] = ctpool.tile([128, NCC], BF16,
                                          name=f"P{bb}", tag="P")
                xs = emit_x(0, spread=True)
                # rest of the weight columns (cf2..7) after slab-0 x
                for i in range(NPR):
                    nc.sync.dma_start(out=w1v[i][:, :, 256:NF],
                                      in_=w1_d[i].rearrange(
                                          "p (k m) -> p k m",
                                          k=2)[:, :, 256:NF])
                nc.sync.dma_start(out=selsb, in_=sel_d)
                nc.sync.dma_start(out=brslsb, in_=brsl_d)
                nc.sync.dma_start(out=outcsb, in_=outc_d)
                nc.sync.dma_start(out=out_d, in_=outcsb)
                # warmup: ~3us of dummy PE work to reach full clock before
                # the first real matmul
                wps = pspool.tile([128, 1024], F32, name="warm", tag="mm")
                for i in range(30):
                    nc.tensor.matmul(wps[0:128, 0:128], lhsT=wscr, rhs=wscr,
                                     start=True, stop=True)

                last = NBB - 1
                for bb in range(NBB):
                    if bb < last:
                        emit_exact(bb, xs, range(0, 4))
                        if bb > 0:
                            emit_sel_mm(bb - 1, range(4))
                            emit_check(bb - 1)
                        for cf in range(4, NCF):
                            # trees alternate between Pool and DVE
                            eng = nc.gpsimd if (cf + bb) % 2 else nc.vector
                            emit_cert_chunk(bb, xs, cf, eng)
                        if bb > 0:
                            # delayed one slab so the W matmul never stalls PE
                            emit_cert_tail(bb - 1)
                        xs = emit_x(bb + 1)
                    else:
                        # final slab: certificate half first, then the exact
                        # scans chased by small selector slices
                        for cf in range(4, NCF):
                            eng = nc.gpsimd if (cf + bb) % 2 else nc.vector
                            emit_cert_chunk(bb, xs, cf, eng)
                        emit_sel_mm(bb - 1, range(4))
                        emit_check(bb - 1)
                        emit_cert_tail(bb - 1)
                        emit_exact(bb, xs, [0, 1])
                        emit_sel_mm(bb, [0])
                        emit_exact(bb, xs, [2])
                        emit_sel_mm(bb, [1])
                        emit_exact(bb, xs, [3])
                        emit_sel_mm(bb, [2])
                        emit_sel_mm(bb, [3])
                        emit_check(bb)
                        emit_cert_tail(bb)

            nc.sync.dma_start(out=flag_d, in_=cnt)

    nc.compile()
    return nc


# ---------------------------------------------------------------------------
# general fallback kernel (sequential spike-correction), used only when the
# no-spike certificate fails: identical to the reference recurrence.
# ---------------------------------------------------------------------------

def build_nc_slow():
    nc = bacc.Bacc("TRN2", target_bir_lowering=False, debug=False,
                   num_devices=N_CORES)
    dt = nc.dram_tensor
    xt_d = dt("xt", [IC * 128, BL, T], BF16, kind="ExternalInput").ap()
    w1_d = dt("w1p", [IC * 128, NF], BF16, kind="ExternalInput").ap()
    w2_d = dt("w2p", [H, NF], BF16, kind="ExternalInput").ap()
    wr_d = dt("wrt", [128, 2 * O], BF16, kind="ExternalInput").ap()
    m2b_d = dt("mh2b", [128, 2 * T], BF16, kind="ExternalInput").ap()
    bsl1_d = dt("bsl1", [NCF, 128, NSL], BF16, kind="ExternalInput").ap()
    bsl2_d = dt("bsl2", [NCF, 128, NSL], BF16, kind="ExternalInput").ap()
    asl_d = dt("asl", [128, 4 * NSL], BF16, kind="ExternalInput").ap()
    acol_d = dt("acol", [128, 4], F32, kind="ExternalInput").ap()
    sel_d = dt("selm", [128, 32], BF16, kind="ExternalInput").ap()
    ur_d = dt("ur", [O, T], F32, kind="ExternalInput").ap()
    bru_d = dt("bru", [O, 1], F32, kind="ExternalInput").ap()
    out_d = dt("out", [O, BL], F32, kind="ExternalOutput").ap()
    flag_d = dt("flag", [1, 2], F32, kind="ExternalOutput").ap()

    with tile.TileContext(nc) as tc:
        with tc.tile_pool(name="const", bufs=1) as cpool, \
             tc.tile_pool(name="state", bufs=1) as spool, \
             tc.tile_pool(name="bsl", bufs=1) as bpool, \
             tc.tile_pool(name="xs", bufs=2) as xpool, \
             tc.tile_pool(name="ds", bufs=2) as dpool, \
             tc.tile_pool(name="small", bufs=1) as mpool:

            w1sb = [cpool.tile([128, NF], BF16, name=f"w1sb{i}", tag=f"w1_{i}")
                    for i in range(IC)]
            for i in range(IC):
                nc.sync.dma_start(out=w1sb[i], in_=w1_d[i * 128:(i + 1) * 128, :])
            w2sb = [cpool.tile([128, NF], BF16, name=f"w2sb{i}", tag=f"w2_{i}")
                    for i in range(2)]
            for i in range(2):
                nc.sync.dma_start(out=w2sb[i], in_=w2_d[i * 128:(i + 1) * 128, :])
            wrsb = cpool.tile([128, 2 * O], BF16, name="wrsb")
            nc.sync.dma_start(out=wrsb, in_=wr_d)
            m2bsb = cpool.tile([128, 2 * T], BF16, name="m2bsb")
            nc.sync.dma_start(out=m2bsb, in_=m2b_d)
            aslsb = cpool.tile([128, 4 * NSL], BF16, name="aslsb")
            nc.sync.dma_start(out=aslsb, in_=asl_d)
            acolsb = cpool.tile([128, 4], F32, name="acolsb")
            nc.sync.dma_start(out=acolsb, in_=acol_d)
            selsb = cpool.tile([128, 32], BF16, name="selsb")
            nc.sync.dma_start(out=selsb, in_=sel_d)
            ursb = cpool.tile([O, T], F32, name="ursb")
            nc.sync.dma_start(out=ursb, in_=ur_d)
            brusb = cpool.tile([O, 1], F32, name="brusb")
            nc.sync.dma_start(out=brusb, in_=bru_d)

            mhat = spool.tile([128, 2 * NBB * NSL], BF16, name="mhat")
            sfull = spool.tile([128, 2 * NBB * NSL], BF16, name="sfull")
            q = mpool.tile([128, 64], BF16, name="q")
            cnt = mpool.tile([128, 4], F32, name="cnt")
            csum = mpool.tile([128, 2], F32, name="csum")
            par = mpool.tile([128, 2], F32, name="par")
            acc = mpool.tile([O, BL], F32, name="acc")
            accb = mpool.tile([O, BL], F32, name="accb")
            zjunk = mpool.tile([O, T], F32, name="zjunk")

            mh_v = mhat.rearrange("p (hh b t) -> p hh b t", hh=2, b=BL, t=T)
            sf_v = sfull.rearrange("p (hh b t) -> p hh b t", hh=2, b=BL, t=T)
            q_v = q.rearrange("p (hh b) -> p hh b", hh=2)

            with tc.tile_pool(name="psA", bufs=2, space="PSUM") as pspool:

                def layer(L, bsl_d, rhs_mm):
                    bslsb = bpool.tile([128, NCF * NSL], BF16, name=f"bslsb{L}",
                                       tag="bsl")
                    for cf in range(NCF):
                        nc.sync.dma_start(out=bslsb[:, cf * NSL:(cf + 1) * NSL],
                                          in_=bsl_d[cf])
                    aoff = (L - 1) * 2 * NSL
                    for bb in range(NBB):
                        ds = dpool.tile([128, NCF * NSL], BF16,
                                        name=f"ds{L}_{bb}", tag="ds")
                        for cf in range(NCF):
                            ps = pspool.tile([128, NSL], F32,
                                             name=f"c{L}_{bb}_{cf}", tag="mm")
                            for nn in range(2):
                                rhs_mm(ps, bb, cf, nn)
                            nc.vector.tensor_tensor_scan(
                                out=ds[:, cf * NSL:(cf + 1) * NSL],
                                data0=bslsb[:, cf * NSL:(cf + 1) * NSL],
                                data1=ps,
                                initial=0.0, op0=ALU.mult, op1=ALU.add)
                        for hh in range(2):
                            Dps = pspool.tile([128, 1024], F32,
                                              name=f"D{L}_{bb}_{hh}", tag="D")
                            for c4 in range(4):
                                o4 = (hh * 4 + c4) * NSL
                                for n0, nw in NN_SPLITS:
                                    nc.tensor.matmul(
                                        Dps[c4 * 32:(c4 + 1) * 32,
                                            n0:n0 + nw],
                                        lhsT=selsb,
                                        rhs=ds[:, o4 + n0:o4 + n0 + nw],
                                        start=True, stop=True,
                                        tile_position=(0, c4 * 32))
                            nc.vector.tensor_tensor_scan(
                                out=mhat[:, hh * 8000 + bb * NSL:
                                         hh * 8000 + (bb + 1) * NSL],
                                data0=aslsb[:, aoff + hh * NSL:
                                            aoff + (hh + 1) * NSL],
                                data1=Dps[:, 0:NSL], initial=0.0,
                                op0=ALU.mult, op1=ALU.add)

                def spike_phase(L):
                    nc.gpsimd.memset(sfull, 0.0)
                    junk = dpool.tile([128, NCF * NSL], BF16,
                                      name=f"junk{L}", tag="ds")
                    for hh in range(2):
                        nc.vector.tensor_scalar(
                            out=junk[:, 0:8000],
                            in0=mhat[:, hh * 8000:(hh + 1) * 8000],
                            scalar1=float(VTH), scalar2=None, op0=ALU.is_gt,
                            op1=ALU.add,
                            accum_out=cnt[:, (L - 1) * 2 + hh:(L - 1) * 2 + hh + 1])
                    nc.vector.tensor_add(
                        out=csum[:, L - 1:L],
                        in0=cnt[:, (L - 1) * 2:(L - 1) * 2 + 1],
                        in1=cnt[:, (L - 1) * 2 + 1:(L - 1) * 2 + 2])
                    nc.gpsimd.partition_all_reduce(
                        par[:, L - 1:L], csum[:, L - 1:L], channels=128,
                        reduce_op=bass_isa.ReduceOp.add)
                    nc.vector.memset(q, 0.0)
                    for t in range(T):
                        nc.vector.scalar_tensor_tensor(
                            out=sf_v[:, :, :, t], in0=mh_v[:, :, :, t],
                            scalar=float(VTH), op0=ALU.subtract,
                            in1=q_v, op1=ALU.is_gt)
                        for hh in range(2):
                            nc.vector.scalar_tensor_tensor(
                                out=q[:, hh * 32:(hh + 1) * 32],
                                in0=q[:, hh * 32:(hh + 1) * 32],
                                scalar=acolsb[:, (L - 1) * 2 + hh:
                                              (L - 1) * 2 + hh + 1],
                                op0=ALU.mult,
                                in1=sf_v[:, hh, :, t], op1=ALU.add)

                xs = {}

                def mm1(ps, bb, cf, nn):
                    n0, nw = NN_SPLITS[nn]
                    if cf == 0 and nn == 0:
                        for i in range(IC):
                            t_ = xpool.tile([128, NSL], BF16,
                                            name=f"xs{bb}_{i}", tag=f"xs{i}")
                            nc.sync.dma_start(
                                out=t_.rearrange("p (b t) -> p b t", b=BBLK),
                                in_=xt_d[i * 128:(i + 1) * 128,
                                         bb * BBLK:(bb + 1) * BBLK, :])
                            xs[i] = t_
                    for i in range(IC):
                        nc.tensor.matmul(
                            ps[:, n0:n0 + nw],
                            lhsT=w1sb[i][:, cf * 128:(cf + 1) * 128],
                            rhs=xs[i][:, n0:n0 + nw],
                            start=(i == 0), stop=(i == IC - 1))

                layer(1, bsl1_d, mm1)
                spike_phase(1)

                def mm2(ps, bb, cf, nn):
                    n0, nw = NN_SPLITS[nn]
                    for hh in range(2):
                        nc.tensor.matmul(
                            ps[:, n0:n0 + nw],
                            lhsT=w2sb[hh][:, cf * 128:(cf + 1) * 128],
                            rhs=sfull[:, hh * 8000 + bb * NSL + n0:
                                      hh * 8000 + bb * NSL + n0 + nw],
                            start=(hh == 0), stop=(hh == 1))

                layer(2, bsl2_d, mm2)
                nc.vector.tensor_add(
                    out=mh_v, in0=mh_v,
                    in1=m2bsb.rearrange("p (hh t) -> p hh t", hh=2)
                        .unsqueeze(2).broadcast_to((128, 2, BL, T)))
                spike_phase(2)

            with tc.tile_pool(name="psB", bufs=2, space="PSUM") as zpool:
                for bb in range(NBB):
                    for nn in range(2):
                        zps = zpool.tile([O, 500], F32, name=f"z{bb}_{nn}",
                                         tag="z")
                        for hh in range(2):
                            nc.tensor.matmul(
                                zps,
                                lhsT=wrsb[:, hh * O:(hh + 1) * O],
                                rhs=sfull[:, hh * 8000 + bb * NSL + nn * 500:
                                          hh * 8000 + bb * NSL + (nn + 1) * 500],
                                start=(hh == 0), stop=(hh == 1))
                        for b2 in range(2):
                            b = bb * BBLK + nn * 2 + b2
                            nc.vector.scalar_tensor_tensor(
                                out=zjunk, in0=zps[:, b2 * T:(b2 + 1) * T],
                                scalar=1.0, op0=ALU.mult,
                                in1=ursb, op1=ALU.mult,
                                accum_out=acc[:, b:b + 1])
                nc.vector.tensor_scalar(
                    out=accb, in0=acc, scalar1=brusb[:, 0:1], scalar2=None,
                    op0=ALU.add)
                nc.sync.dma_start(out=out_d, in_=accb)
                nc.sync.dma_start(out=flag_d, in_=par[0:1, 0:2])

    nc.compile()
    return nc


_NC_CACHE = {}


def get_nc():
    if "fast" not in _NC_CACHE:
        _NC_CACHE["fast"] = build_nc()
    return _NC_CACHE["fast"]


def get_nc_slow():
    if "slow" not in _NC_CACHE:
        _NC_CACHE["slow"] = build_nc_slow()
    return _NC_CACHE["slow"]


def prep_inputs(x, W1, b1, tau_n1, tau_m1, W2, b2, tau_n2, tau_m2,
                Wr, br, tau_mr, warmup):
    """Host-side: per-core input dicts for the fast bass kernel, plus the
    host-verified layer-2/readout constants. Returns (in_maps, fast_ok)."""
    w = int(np.asarray(warmup))
    beta1 = _sig(tau_n1).reshape(NF)          # [H,K], j = h*4+k order
    alpha1 = _sig(tau_m1)                     # [H]
    beta2 = _sig(tau_n2).reshape(NF)
    alpha2 = _sig(tau_m2)
    alphar = _sig(tau_mr)                     # [O]

    g1 = (1.0 - beta1) * np.repeat(1.0 - alpha1, K)

    # fp8 weights, prescaled by WSC; row 700 = bias, rows 701.. = 0
    w1t = np.zeros((IC * 128, NF), np.float64)
    w1t[:IN] = np.asarray(W1, np.float64).T * WSC
    w1t[IN] = np.asarray(b1, np.float64) * WSC
    w1q = np.empty((NPR, 128, 2 * NF), ml_dtypes.float8_e4m3)
    for pr in range(NPR):
        w1q[pr, :, :NF] = w1t[2 * pr * 128:(2 * pr + 1) * 128]
        w1q[pr, :, NF:] = w1t[(2 * pr + 1) * 128:(2 * pr + 2) * 128]

    # selector: g/WSC weights, [128, 32] blocks per feature chunk, packed
    selm = np.zeros((128, NCF * 32), ml_dtypes.bfloat16)
    for cf in range(NCF):
        j = cf * 128 + np.arange(128)
        selm[np.arange(128), cf * 32 + np.arange(128) // 4] = g1[j] / WSC

    betacol = np.ascontiguousarray(
        beta1.reshape(NCF, 128).T[:, 0:4].astype(np.float32))
    # beta^RBLK slab for the certificate half (cf 4-7), zero at block 0 of
    # each (chunk, batch) stream
    brs = np.zeros((128, 4, BBLK, NBLK), np.float64)
    brs[:] = (beta1.reshape(NCF, 128).T[:, 4:8] ** RBLK)[:, :, None, None]
    brs[:, :, :, 0] = 0.0
    brsl = brs.reshape(128, NCC).astype(ml_dtypes.bfloat16)

    # host-exact layer-2 bias trajectory (valid when layer 1 has no spikes)
    b2g = np.asarray(b2, np.float64) * (1.0 - beta2)
    dtraj = np.zeros(NF)
    mtraj = np.zeros(H)
    m2max = -np.inf
    for _ in range(T):
        dtraj = beta2 * dtraj + b2g
        mtraj = alpha2 * mtraj + (1.0 - alpha2) * dtraj.reshape(H, K).sum(-1)
        m2max = max(m2max, mtraj.max())
    fast_ok = bool(m2max <= VTH - 0.05)

    # host-exact readout constant (valid when layer 2 has no spikes)
    mr = np.zeros(O)
    accr = np.zeros(O)
    for t_ in range(T):
        mr = mr * alphar + (1.0 - alphar) * np.asarray(br, np.float64)
        if t_ >= w:
            accr += mr
    outc = np.tile((accr / (T - w)).astype(np.float32)[:, None], (1, BL))

    xq_full = np.zeros((IC * 128, B, T), ml_dtypes.float8_e4m3)
    xq_full[:IN] = np.asarray(x).transpose(2, 0, 1)
    xq_full[IN] = 1.0
    # pair-interleaved: [NPR, 128, 2, B, T]
    xq_full = np.ascontiguousarray(
        xq_full.reshape(NPR, 2, 128, B, T).transpose(0, 2, 1, 3, 4))

    shared = dict(w1q=w1q, selm=selm, betacol=betacol, brsl=brsl, outc=outc)
    in_maps = []
    for c in range(N_CORES):
        m = dict(shared)
        m["xq"] = np.ascontiguousarray(
            xq_full[:, :, :, c * BL:(c + 1) * BL, :])
        in_maps.append(m)
    return in_maps, fast_ok


def prep_inputs_slow(x, W1, b1, tau_n1, tau_m1, W2, b2, tau_n2, tau_m2,
                     Wr, br, tau_mr, warmup):
    """Host-side prep for the general fallback kernel."""
    w = int(np.asarray(warmup))
    beta1 = _sig(tau_n1).reshape(NF)
    alpha1 = _sig(tau_m1)
    beta2 = _sig(tau_n2).reshape(NF)
    alpha2 = _sig(tau_m2)
    alphar = _sig(tau_mr)

    g1 = (1.0 - beta1) * np.repeat(1.0 - alpha1, K)
    g2 = (1.0 - beta2) * np.repeat(1.0 - alpha2, K)

    w1p = np.zeros((IC * 128, NF), np.float64)
    w1p[:IN] = np.asarray(W1, np.float64).T * g1
    w1p[IN] = np.asarray(b1, np.float64) * g1
    w1p = w1p.astype(ml_dtypes.bfloat16)

    w2p = (np.asarray(W2, np.float64).T * g2).astype(ml_dtypes.bfloat16)
    b2g = np.asarray(b2, np.float64) * g2
    dtraj = np.zeros(NF)
    mh2b = np.zeros((H, T))
    mtraj = np.zeros(H)
    for t_ in range(T):
        dtraj = _sig(tau_n2).reshape(NF) * dtraj + b2g
        mtraj = _sig(tau_m2) * mtraj + dtraj.reshape(H, K).sum(-1)
        mh2b[:, t_] = mtraj
    mh2b_dev = np.zeros((128, 2 * T), np.float64)
    mh2b_dev[:, :T] = mh2b[:128]
    mh2b_dev[:, T:] = mh2b[128:]
    mh2b_dev = mh2b_dev.astype(ml_dtypes.bfloat16)

    wrt = np.zeros((128, 2 * O), np.float64)
    wrt[:, :O] = np.asarray(Wr, np.float64).T[:128]
    wrt[:, O:] = np.asarray(Wr, np.float64).T[128:]
    wrt = wrt.astype(ml_dtypes.bfloat16)

    def bslab(beta):
        s = np.tile(beta.reshape(NCF, 128, 1).astype(ml_dtypes.bfloat16),
                    (1, 1, NSL))
        s.reshape(NCF, 128, BBLK, T)[:, :, :, 0] = 0.0
        return s

    bsl1 = bslab(beta1)
    bsl2 = bslab(beta2)

    def aslab(alpha):
        a2 = alpha.reshape(2, 128).astype(ml_dtypes.bfloat16)
        s = np.tile(a2[:, :, None], (1, 1, NSL))
        s.reshape(2, 128, BBLK, T)[:, :, :, 0] = 0.0
        return s

    asl = np.concatenate([aslab(alpha1), aslab(alpha2)], axis=0)
    asl = asl.transpose(1, 0, 2).reshape(128, 4 * NSL).copy()

    acol = np.stack([alpha1[:128], alpha1[128:], alpha2[:128], alpha2[128:]],
                    axis=1).astype(np.float32)

    selm = np.zeros((128, 32), ml_dtypes.bfloat16)
    selm[np.arange(128), np.arange(128) // 4] = 1.0

    tt = np.arange(T, dtype=np.float64)[:, None]
    ar = alphar[None, :]
    u = ar ** np.maximum(0, w - tt) - ar ** (T - tt)
    ur = (u.T / (T - w)).astype(np.float32)
    bru = (np.asarray(br, np.float64) * u.sum(0) / (T - w)) \
        .astype(np.float32)[:, None]

    xt_full = np.zeros((IC * 128, B, T), ml_dtypes.bfloat16)
    xt_full[:IN] = np.asarray(x).transpose(2, 0, 1)
    xt_full[IN] = 1.0

    shared = dict(w1p=w1p, w2p=w2p, mh2b=mh2b_dev, wrt=wrt,
                  bsl1=bsl1, bsl2=bsl2, asl=asl, acol=acol, selm=selm,
                  ur=ur, bru=bru)
    in_maps = []
    for c in range(N_CORES):
        m = dict(shared)
        m["xt"] = np.ascontiguousarray(xt_full[:, c * BL:(c + 1) * BL, :])
        in_maps.append(m)
    return in_maps


def _run_slow(**inputs):
    in_maps = prep_inputs_slow(**inputs)
    res = bass_utils.run_bass_kernel_spmd(
        get_nc_slow(), in_maps, core_ids=list(range(N_CORES)))
    out = np.empty((B, O), np.float32)
    for c in range(N_CORES):
        out[c * BL:(c + 1) * BL] = res.results[c]["out"].T
    return out


def kernel(**inputs):
    in_maps, fast_ok = prep_inputs(**inputs)
    if not fast_ok:
        return _run_slow(**inputs)
    res = bass_utils.run_bass_kernel_spmd(
        get_nc(), in_maps, core_ids=list(range(N_CORES)))
    if any(r["flag"].sum() > 0 for r in res.results):
        # certificate failed: spikes may exist, use the general kernel
        return _run_slow(**inputs)
    out = np.empty((B, O), np.float32)
    for c in range(N_CORES):
        out[c * BL:(c + 1) * BL] = res.results[c]["out"].T
    return out


# revision 38
# speedup vs baseline: 1.0942x; 1.0103x over previous
"""DH-SFNN Trainium2 kernel (8 NeuronCores, data-parallel over batch).

Model: 2 dendritic LIF layers (K=4 branches, reset-by-subtraction) + leaky
readout integrator, T=250 steps, B=256, IN=700, H=256, O=20.

Fast path (per core, B_l=32), exploiting reset-by-subtraction soundness:
spike corrections are strictly subtractive, so if the no-spike layer-1
membrane trajectory m1^ satisfies max m1^ <= VTH there are exactly zero
layer-1 spikes. Layer 2 then sees only its bias trajectory (x-independent,
verified exactly on host), and the readout is a batch-independent constant
computed on host. The device therefore only needs to certify layer 1:

    c1 = x @ (16*W1).T (+bias row)     -- fp8 DoubleRow matmuls (2x128
                                          contraction rows per instr)
    d1 = per-feature 1-pole IIR over t -- DVE tensor_tensor_scan, 4 batch
                                          streams packed per instruction with
                                          zeroed-multiplier boundary columns
    D1 = sum_k g_k d1_k               -- PE matmul with g/16-weighted selector
    check max_t D1 <= VTH - 0.25      -- Act engine relu-accumulate; since
                                          m1^ is a running convex combination
                                          of D1, max m1^ <= max(0, max D1).

If the on-device flag fires, or the host-side layer-2 bias-trajectory check
fails, rerun with the general sequential-correction kernel (slow path).
"""
import sys

sys.path.insert(0, "/opt/trn_rl_repo")

import numpy as np
import ml_dtypes

import concourse.bass as bass
import concourse.mybir as mybir
import concourse.tile as tile
from concourse import bacc, bass_utils, bass_isa

F32 = mybir.dt.float32
BF16 = mybir.dt.bfloat16
FP8 = mybir.dt.float8e4
ALU = mybir.AluOpType
ACT = mybir.ActivationFunctionType
DR = mybir.MatmulPerfMode.DoubleRow

N_CORES = 8
B, T, IN, H, O, K = 256, 250, 700, 256, 20, 4
BL = B // N_CORES            # 32 batch per core
BBLK = 4                     # batches per scan slab
NBB = BL // BBLK             # 8 slabs
NSL = BBLK * T               # 1000 slab columns
IC = 6                       # 768 = 6*128 contraction rows (row 700 = bias)
NPR = IC // 2                # 3 DoubleRow pair chunks
NF = H * K                   # 1024 layer-1 branch features
NCF = NF // 128              # 8 feature chunks
VTH = 1.0
CHECK_MARGIN = 0.25          # device certifies max D <= VTH - margin
WSC = 16.0                   # power-of-2 prescale on W1 for fp8 range
RBLK = 10                    # certificate block length (T = 25 blocks)
NBLK = T // RBLK
NCC = 4 * BBLK * NBLK        # certificate cols per slab: 4 chunks x 4b x 25m
# out-column splits of the 1000 slab columns, each within one PSUM bank
CSPLITS = [(0, 256), (256, 256), (512, 256), (768, 232)]
NN_SPLITS = [(0, 512), (512, 488)]


def _sig(v):
    return 1.0 / (1.0 + np.exp(-np.asarray(v, np.float64)))


def build_nc():
    nc = bacc.Bacc("TRN2", target_bir_lowering=False, debug=False,
                   num_devices=N_CORES)
    dt = nc.dram_tensor
    xq_d = dt("xq", [NPR, 128, 2, BL, T], FP8, kind="ExternalInput").ap()
    w1_d = dt("w1q", [NPR, 128, 2 * NF], FP8, kind="ExternalInput").ap()
    sel_d = dt("selm", [128, NCF * 32], BF16, kind="ExternalInput").ap()
    bcol_d = dt("betacol", [128, NCF // 2], F32, kind="ExternalInput").ap()
    brsl_d = dt("brsl", [128, 4 * BBLK * NBLK], BF16,
                kind="ExternalInput").ap()
    outc_d = dt("outc", [O, BL], F32, kind="ExternalInput").ap()
    out_d = dt("out", [O, BL], F32, kind="ExternalOutput").ap()
    flag_d = dt("flag", [128, 2 * NBB], F32, kind="ExternalOutput").ap()

    with tile.TileContext(nc) as tc:
        with tc.tile_pool(name="const", bufs=1) as cpool, \
             tc.tile_pool(name="xs", bufs=2) as xpool, \
             tc.tile_pool(name="ds", bufs=2) as dpool, \
             tc.tile_pool(name="small", bufs=1) as mpool:

            # ---- constants ----
            # wire order: betacol, w1 cf0/1-columns, x slab 0, w1 rest.
            # beta slabs (exact half, cf0-3) are built on the Act engine
            # (mask * beta-column) keeping the serial DMA wire free for x.
            w1sb = [cpool.tile([128, 2 * NF], FP8, name=f"w1sb{i}",
                               tag=f"w1_{i}") for i in range(NPR)]
            bslsb = cpool.tile([128, 4 * NSL], BF16, name="bslsb")
            bcolsb = cpool.tile([128, NCF // 2], F32, name="bcolsb")
            brslsb = cpool.tile([128, NCC], BF16, name="brslsb")
            nc.sync.dma_start(out=bcolsb, in_=bcol_d)
            w1v = [w.rearrange("p (k m) -> p k m", k=2) for w in w1sb]
            for i in range(NPR):
                nc.sync.dma_start(out=w1v[i][:, :, 0:256],
                                  in_=w1_d[i].rearrange("p (k m) -> p k m",
                                                        k=2)[:, :, 0:256])
            selsb = cpool.tile([128, NCF * 32], BF16, name="selsb")
            outcsb = cpool.tile([O, BL], F32, name="outcsb")
            biasc = mpool.tile([128, 1], F32, name="biasc")
            nc.vector.memset(biasc, -(VTH - CHECK_MARGIN))
            # certificate checks sum_k g*u against (VTH - margin)/2 since
            # max_t D in block m <= u[m-1] + P[m] <= 2 max_m u[m]
            bias2 = mpool.tile([128, 1], F32, name="bias2")
            nc.vector.memset(bias2, -(VTH - CHECK_MARGIN) / 2)
            biasz = mpool.tile([128, 1], F32, name="biasz")
            nc.vector.memset(biasz, 0.0)
            # PE p-state warmup: dummy matmuls on a zeroed scratch tile keep
            # the tensor engine ramping while the first DMAs land.
            wscr = mpool.tile([128, 128], BF16, name="wscr")
            nc.vector.memset(wscr, 0.0)
            # mask for the beta slabs: ones, zero at each batch-stream start
            mask = mpool.tile([128, NSL], BF16, name="mask")
            nc.vector.memset(mask, 1.0)
            for b_ in range(BBLK):
                nc.vector.memset(mask[:, b_ * T:b_ * T + 1], 0.0)
            for cf in range(4):
                nc.scalar.activation(out=bslsb[:, cf * NSL:(cf + 1) * NSL],
                                     in_=mask, func=ACT.Copy, bias=0.0,
                                     scale=bcolsb[:, cf:cf + 1])

            cnt = mpool.tile([128, 2 * NBB], F32, name="cnt")
            junk = mpool.tile([128, NSL], BF16, name="junk")

            with tc.tile_pool(name="psA", bufs=2, space="PSUM") as pspool, \
                 tc.tile_pool(name="psB", bufs=2, space="PSUM") as dppool, \
                 tc.tile_pool(name="rc", bufs=2) as rcpool, \
                 tc.tile_pool(name="ct", bufs=2) as ctpool:
                dss = {}
                rcs = {}
                Pts = {}

                def emit_x(bb, spread=False):
                    xs = []
                    eng = [nc.sync] * NPR
                    if spread:
                        eng = [nc.sync, nc.sync, nc.gpsimd]
                    for pr in range(NPR):
                        t_ = xpool.tile([128, 2 * NSL], FP8,
                                        name=f"xs{bb}_{pr}", tag=f"xs{pr}")
                        eng[pr].dma_start(
                            out=t_.rearrange("p (k b t) -> p k b t",
                                             k=2, b=BBLK),
                            in_=xq_d[pr][:, :, bb * BBLK:(bb + 1) * BBLK, :])
                        xs.append(t_.rearrange("p (k n) -> p k n", k=2))
                    return xs

                def emit_cmm(bb, xs, cf):
                    ps = pspool.tile([128, 1024], F32,
                                     name=f"c{bb}_{cf}", tag="mm")
                    for n0, nw in CSPLITS:
                        for pr in range(NPR):
                            nc.tensor.matmul(
                                ps[:, n0:n0 + nw],
                                lhsT=w1sb[pr]
                                    .rearrange("p (k m) -> p k m", k=2)
                                    [:, :, cf * 128:(cf + 1) * 128],
                                rhs=xs[pr][:, :, n0:n0 + nw],
                                start=(pr == 0), stop=(pr == NPR - 1),
                                perf_mode=DR)
                    return ps

                def emit_exact(bb, xs, cfs):
                    # exact path (cf 0-3): beta-IIR scan on DVE
                    ds = dss[bb]
                    for cf in cfs:
                        ps = emit_cmm(bb, xs, cf)
                        nc.vector.tensor_tensor_scan(
                            out=ds[:, cf * NSL:(cf + 1) * NSL],
                            data0=bslsb[:, cf * NSL:(cf + 1) * NSL],
                            data1=ps[:, 0:NSL], initial=0.0,
                            op0=ALU.mult, op1=ALU.add)

                def emit_cert_relu(bb, xs, cf):
                    # certificate path (cf 4-7): relu(c) psum -> sbuf on Act
                    cc = cf - 4
                    rc = rcs[bb]
                    nc.scalar.activation(
                        out=rc[:, cc * NSL:(cc + 1) * NSL],
                        in_=emit_cmm(bb, xs, cf)[:, 0:NSL],
                        func=ACT.Relu, bias=biasz, scale=1.0)

                def emit_cert_tree(bb, cf, tree_eng):
                    # block sums of relu(c) over RBLK columns
                    cc = cf - 4
                    rc = rcs[bb]
                    scr = rcs[bb + 100]
                    sv = scr[:, cc * 500:(cc + 1) * 500] \
                        .rearrange("p (s r) -> p s r", s=100)
                    rv = rc[:, cc * NSL:(cc + 1) * NSL] \
                        .rearrange("p (s r) -> p s r", s=100)
                    Pv = Pts[bb][:, cc * 100:(cc + 1) * 100]
                    E = tree_eng
                    E.tensor_tensor(out=sv, in0=rv[:, :, 0:5],
                                    in1=rv[:, :, 5:10], op=ALU.add)
                    E.tensor_tensor(out=sv[:, :, 0:2], in0=sv[:, :, 0:2],
                                    in1=sv[:, :, 2:4], op=ALU.add)
                    E.tensor_tensor(out=sv[:, :, 0:1], in0=sv[:, :, 0:1],
                                    in1=sv[:, :, 1:2], op=ALU.add)
                    E.tensor_tensor(out=Pv[:, :, None], in0=sv[:, :, 0:1],
                                    in1=sv[:, :, 4:5], op=ALU.add)

                def emit_cert_tail(bb):
                    # u-scan over blocks, selector matmul on u, check
                    Pt = Pts[bb]
                    ut = ctpool.tile([128, NCC], BF16, name=f"u{bb}", tag="u")
                    nc.vector.tensor_tensor_scan(
                        out=ut, data0=brslsb, data1=Pt, initial=0.0,
                        op0=ALU.mult, op1=ALU.add)
                    Wps = dppool.tile([128, 1024], F32,
                                      name=f"W{bb}", tag="D")
                    for c4 in range(4):
                        cf = 4 + c4
                        nc.tensor.matmul(
                            Wps[c4 * 32:(c4 + 1) * 32, 0:100],
                            lhsT=selsb[:, cf * 32:(cf + 1) * 32],
                            rhs=ut[:, c4 * 100:(c4 + 1) * 100],
                            start=True, stop=True,
                            tile_position=(0, c4 * 32))
                    nc.scalar.activation(
                        out=junk[:, 0:100], in_=Wps[:, 0:100], func=ACT.Relu,
                        bias=bias2, scale=1.0,
                        accum_out=cnt[:, bb * 2 + 1:bb * 2 + 2])

                Dcur = {}

                def emit_sel_mm(bb, c4s):
                    ds = dss[bb]
                    if bb not in Dcur:
                        Dcur[bb] = dppool.tile(
                            [128, 1024], F32, name=f"D{bb}", tag="D")
                    Dps = Dcur[bb]
                    for c4 in c4s:
                        for n0, nw in NN_SPLITS:
                            nc.tensor.matmul(
                                Dps[c4 * 32:(c4 + 1) * 32, n0:n0 + nw],
                                lhsT=selsb[:, c4 * 32:(c4 + 1) * 32],
                                rhs=ds[:, c4 * NSL + n0:c4 * NSL + n0 + nw],
                                start=True, stop=True,
                                tile_position=(0, c4 * 32))

                def emit_check(bb):
                    # exact half: relu(D - (VTH - margin)) summed
                    Dps = Dcur.pop(bb)
                    nc.scalar.activation(
                        out=junk, in_=Dps[:, 0:NSL], func=ACT.Relu,
                        bias=biasc, scale=1.0,
                        accum_out=cnt[:, bb * 2:bb * 2 + 1])

                for bb in range(NBB):
                    dss[bb] = dpool.tile([128, 4 * NSL], BF16,
                                         name=f"ds{bb}", tag="ds")
                    rcs[bb] = rcpool.tile([128, 4 * NSL], BF16,
                                          name=f"rc{bb}", tag="rc")
                    rcs[bb + 100] = rcpool.tile([128, 4 * 500], BF16,
                                                name=f"scr{bb}", tag="scr")
                    Pts[bb] = ctpool.tile([128, NCC], BF16,
                                          name=f"P{bb}", tag="P")
                xs = emit_x(0, spread=True)
                # rest of the weight columns (cf2..7) after slab-0 x
                for i in range(NPR):
                    nc.sync.dma_start(out=w1v[i][:, :, 256:NF],
                                      in_=w1_d[i].rearrange(
                                          "p (k m) -> p k m",
                                          k=2)[:, :, 256:NF])
                nc.sync.dma_start(out=selsb, in_=sel_d)
                nc.sync.dma_start(out=brslsb, in_=brsl_d)
                nc.sync.dma_start(out=outcsb, in_=outc_d)
                nc.sync.dma_start(out=out_d, in_=outcsb)
                # warmup: ~3us of dummy PE work to reach full clock before
                # the first real matmul
                wps = pspool.tile([128, 1024], F32, name="warm", tag="mm")
                for i in range(30):
                    nc.tensor.matmul(wps[0:128, 0:128], lhsT=wscr, rhs=wscr,
                                     start=True, stop=True)

                last = NBB - 1
                for bb in range(NBB):
                    if bb < last:
                        # interleave cert (Act/Pool) and exact (DVE) chunks
                        # so every engine is fed from the top of the slab;
                        # Pool trees (cf4/5) fire early, DVE trees (cf6/7)
                        # are emitted after the scans.
                        emit_cert_relu(bb, xs, 4)
                        emit_exact(bb, xs, [0])
                        emit_cert_tree(bb, 4, nc.gpsimd)
                        emit_cert_relu(bb, xs, 5)
                        if bb > 0:
                            emit_sel_mm(bb - 1, range(4))
                        emit_exact(bb, xs, [1])
                        emit_cert_tree(bb, 5, nc.gpsimd)
                        emit_cert_relu(bb, xs, 6)
                        if bb > 0:
                            emit_check(bb - 1)
                        emit_exact(bb, xs, [2])
                        emit_cert_relu(bb, xs, 7)
                        emit_exact(bb, xs, [3])
                        emit_cert_tree(bb, 6, nc.vector)
                        emit_cert_tree(bb, 7, nc.vector)
                        if bb > 0:
                            # delayed one slab so the W matmul never stalls PE
                            emit_cert_tail(bb - 1)
                        xs = emit_x(bb + 1)
                    else:
                        # final slab: certificate half first, then the exact
                        # scans chased by small selector slices
                        emit_cert_relu(bb, xs, 4)
                        emit_cert_relu(bb, xs, 5)
                        emit_cert_tree(bb, 4, nc.gpsimd)
                        emit_sel_mm(bb - 1, range(4))
                        emit_cert_relu(bb, xs, 6)
                        emit_cert_tree(bb, 5, nc.gpsimd)
                        emit_check(bb - 1)
                        emit_cert_relu(bb, xs, 7)
                        emit_cert_tail(bb - 1)
                        emit_cert_tree(bb, 6, nc.vector)
                        emit_exact(bb, xs, [0, 1])
                        emit_cert_tree(bb, 7, nc.gpsimd)
                        emit_sel_mm(bb, [0])
                        emit_exact(bb, xs, [2])
                        emit_sel_mm(bb, [1])
                        emit_cert_tail(bb)
                        emit_exact(bb, xs, [3])
                        emit_sel_mm(bb, [2])
                        emit_sel_mm(bb, [3])
                        emit_check(bb)

            nc.sync.dma_start(out=flag_d, in_=cnt)

    nc.compile()
    return nc


# ---------------------------------------------------------------------------
# general fallback kernel (sequential spike-correction), used only when the
# no-spike certificate fails: identical to the reference recurrence.
# ---------------------------------------------------------------------------

def build_nc_slow():
    nc = bacc.Bacc("TRN2", target_bir_lowering=False, debug=False,
                   num_devices=N_CORES)
    dt = nc.dram_tensor
    xt_d = dt("xt", [IC * 128, BL, T], BF16, kind="ExternalInput").ap()
    w1_d = dt("w1p", [IC * 128, NF], BF16, kind="ExternalInput").ap()
    w2_d = dt("w2p", [H, NF], BF16, kind="ExternalInput").ap()
    wr_d = dt("wrt", [128, 2 * O], BF16, kind="ExternalInput").ap()
    m2b_d = dt("mh2b", [128, 2 * T], BF16, kind="ExternalInput").ap()
    bsl1_d = dt("bsl1", [NCF, 128, NSL], BF16, kind="ExternalInput").ap()
    bsl2_d = dt("bsl2", [NCF, 128, NSL], BF16, kind="ExternalInput").ap()
    asl_d = dt("asl", [128, 4 * NSL], BF16, kind="ExternalInput").ap()
    acol_d = dt("acol", [128, 4], F32, kind="ExternalInput").ap()
    sel_d = dt("selm", [128, 32], BF16, kind="ExternalInput").ap()
    ur_d = dt("ur", [O, T], F32, kind="ExternalInput").ap()
    bru_d = dt("bru", [O, 1], F32, kind="ExternalInput").ap()
    out_d = dt("out", [O, BL], F32, kind="ExternalOutput").ap()
    flag_d = dt("flag", [1, 2], F32, kind="ExternalOutput").ap()

    with tile.TileContext(nc) as tc:
        with tc.tile_pool(name="const", bufs=1) as cpool, \
             tc.tile_pool(name="state", bufs=1) as spool, \
             tc.tile_pool(name="bsl", bufs=1) as bpool, \
             tc.tile_pool(name="xs", bufs=2) as xpool, \
             tc.tile_pool(name="ds", bufs=2) as dpool, \
             tc.tile_pool(name="small", bufs=1) as mpool:

            w1sb = [cpool.tile([128, NF], BF16, name=f"w1sb{i}", tag=f"w1_{i}")
                    for i in range(IC)]
            for i in range(IC):
                nc.sync.dma_start(out=w1sb[i], in_=w1_d[i * 128:(i + 1) * 128, :])
            w2sb = [cpool.tile([128, NF], BF16, name=f"w2sb{i}", tag=f"w2_{i}")
                    for i in range(2)]
            for i in range(2):
                nc.sync.dma_start(out=w2sb[i], in_=w2_d[i * 128:(i + 1) * 128, :])
            wrsb = cpool.tile([128, 2 * O], BF16, name="wrsb")
            nc.sync.dma_start(out=wrsb, in_=wr_d)
            m2bsb = cpool.tile([128, 2 * T], BF16, name="m2bsb")
            nc.sync.dma_start(out=m2bsb, in_=m2b_d)
            aslsb = cpool.tile([128, 4 * NSL], BF16, name="aslsb")
            nc.sync.dma_start(out=aslsb, in_=asl_d)
            acolsb = cpool.tile([128, 4], F32, name="acolsb")
            nc.sync.dma_start(out=acolsb, in_=acol_d)
            selsb = cpool.tile([128, 32], BF16, name="selsb")
            nc.sync.dma_start(out=selsb, in_=sel_d)
            ursb = cpool.tile([O, T], F32, name="ursb")
            nc.sync.dma_start(out=ursb, in_=ur_d)
            brusb = cpool.tile([O, 1], F32, name="brusb")
            nc.sync.dma_start(out=brusb, in_=bru_d)

            mhat = spool.tile([128, 2 * NBB * NSL], BF16, name="mhat")
            sfull = spool.tile([128, 2 * NBB * NSL], BF16, name="sfull")
            q = mpool.tile([128, 64], BF16, name="q")
            cnt = mpool.tile([128, 4], F32, name="cnt")
            csum = mpool.tile([128, 2], F32, name="csum")
            par = mpool.tile([128, 2], F32, name="par")
            acc = mpool.tile([O, BL], F32, name="acc")
            accb = mpool.tile([O, BL], F32, name="accb")
            zjunk = mpool.tile([O, T], F32, name="zjunk")

            mh_v = mhat.rearrange("p (hh b t) -> p hh b t", hh=2, b=BL, t=T)
            sf_v = sfull.rearrange("p (hh b t) -> p hh b t", hh=2, b=BL, t=T)
            q_v = q.rearrange("p (hh b) -> p hh b", hh=2)

            with tc.tile_pool(name="psA", bufs=2, space="PSUM") as pspool:

                def layer(L, bsl_d, rhs_mm):
                    bslsb = bpool.tile([128, NCF * NSL], BF16, name=f"bslsb{L}",
                                       tag="bsl")
                    for cf in range(NCF):
                        nc.sync.dma_start(out=bslsb[:, cf * NSL:(cf + 1) * NSL],
                                          in_=bsl_d[cf])
                    aoff = (L - 1) * 2 * NSL
                    for bb in range(NBB):
                        ds = dpool.tile([128, NCF * NSL], BF16,
                                        name=f"ds{L}_{bb}", tag="ds")
                        for cf in range(NCF):
                            ps = pspool.tile([128, NSL], F32,
                                             name=f"c{L}_{bb}_{cf}", tag="mm")
                            for nn in range(2):
                                rhs_mm(ps, bb, cf, nn)
                            nc.vector.tensor_tensor_scan(
                                out=ds[:, cf * NSL:(cf + 1) * NSL],
                                data0=bslsb[:, cf * NSL:(cf + 1) * NSL],
                                data1=ps,
                                initial=0.0, op0=ALU.mult, op1=ALU.add)
                        for hh in range(2):
                            Dps = pspool.tile([128, 1024], F32,
                                              name=f"D{L}_{bb}_{hh}", tag="D")
                            for c4 in range(4):
                                o4 = (hh * 4 + c4) * NSL
                                for n0, nw in NN_SPLITS:
                                    nc.tensor.matmul(
                                        Dps[c4 * 32:(c4 + 1) * 32,
                                            n0:n0 + nw],
                                        lhsT=selsb,
                                        rhs=ds[:, o4 + n0:o4 + n0 + nw],
                                        start=True, stop=True,
                                        tile_position=(0, c4 * 32))
                            nc.vector.tensor_tensor_scan(
                                out=mhat[:, hh * 8000 + bb * NSL:
                                         hh * 8000 + (bb + 1) * NSL],
                                data0=aslsb[:, aoff + hh * NSL:
                                            aoff + (hh + 1) * NSL],
                                data1=Dps[:, 0:NSL], initial=0.0,
                                op0=ALU.mult, op1=ALU.add)

                def spike_phase(L):
                    nc.gpsimd.memset(sfull, 0.0)
                    junk = dpool.tile([128, NCF * NSL], BF16,
                                      name=f"junk{L}", tag="ds")
                    for hh in range(2):
                        nc.vector.tensor_scalar(
                            out=junk[:, 0:8000],
                            in0=mhat[:, hh * 8000:(hh + 1) * 8000],
                            scalar1=float(VTH), scalar2=None, op0=ALU.is_gt,
                            op1=ALU.add,
                            accum_out=cnt[:, (L - 1) * 2 + hh:(L - 1) * 2 + hh + 1])
                    nc.vector.tensor_add(
                        out=csum[:, L - 1:L],
                        in0=cnt[:, (L - 1) * 2:(L - 1) * 2 + 1],
                        in1=cnt[:, (L - 1) * 2 + 1:(L - 1) * 2 + 2])
                    nc.gpsimd.partition_all_reduce(
                        par[:, L - 1:L], csum[:, L - 1:L], channels=128,
                        reduce_op=bass_isa.ReduceOp.add)
                    nc.vector.memset(q, 0.0)
                    for t in range(T):
                        nc.vector.scalar_tensor_tensor(
                            out=sf_v[:, :, :, t], in0=mh_v[:, :, :, t],
                            scalar=float(VTH), op0=ALU.subtract,
                            in1=q_v, op1=ALU.is_gt)
                        for hh in range(2):
                            nc.vector.scalar_tensor_tensor(
                                out=q[:, hh * 32:(hh + 1) * 32],
                                in0=q[:, hh * 32:(hh + 1) * 32],
                                scalar=acolsb[:, (L - 1) * 2 + hh:
                                              (L - 1) * 2 + hh + 1],
                                op0=ALU.mult,
                                in1=sf_v[:, hh, :, t], op1=ALU.add)

                xs = {}

                def mm1(ps, bb, cf, nn):
                    n0, nw = NN_SPLITS[nn]
                    if cf == 0 and nn == 0:
                        for i in range(IC):
                            t_ = xpool.tile([128, NSL], BF16,
                                            name=f"xs{bb}_{i}", tag=f"xs{i}")
                            nc.sync.dma_start(
                                out=t_.rearrange("p (b t) -> p b t", b=BBLK),
                                in_=xt_d[i * 128:(i + 1) * 128,
                                         bb * BBLK:(bb + 1) * BBLK, :])
                            xs[i] = t_
                    for i in range(IC):
                        nc.tensor.matmul(
                            ps[:, n0:n0 + nw],
                            lhsT=w1sb[i][:, cf * 128:(cf + 1) * 128],
                            rhs=xs[i][:, n0:n0 + nw],
                            start=(i == 0), stop=(i == IC - 1))

                layer(1, bsl1_d, mm1)
                spike_phase(1)

                def mm2(ps, bb, cf, nn):
                    n0, nw = NN_SPLITS[nn]
                    for hh in range(2):
                        nc.tensor.matmul(
                            ps[:, n0:n0 + nw],
                            lhsT=w2sb[hh][:, cf * 128:(cf + 1) * 128],
                            rhs=sfull[:, hh * 8000 + bb * NSL + n0:
                                      hh * 8000 + bb * NSL + n0 + nw],
                            start=(hh == 0), stop=(hh == 1))

                layer(2, bsl2_d, mm2)
                nc.vector.tensor_add(
                    out=mh_v, in0=mh_v,
                    in1=m2bsb.rearrange("p (hh t) -> p hh t", hh=2)
                        .unsqueeze(2).broadcast_to((128, 2, BL, T)))
                spike_phase(2)

            with tc.tile_pool(name="psB", bufs=2, space="PSUM") as zpool:
                for bb in range(NBB):
                    for nn in range(2):
                        zps = zpool.tile([O, 500], F32, name=f"z{bb}_{nn}",
                                         tag="z")
                        for hh in range(2):
                            nc.tensor.matmul(
                                zps,
                                lhsT=wrsb[:, hh * O:(hh + 1) * O],
                                rhs=sfull[:, hh * 8000 + bb * NSL + nn * 500:
                                          hh * 8000 + bb * NSL + (nn + 1) * 500],
                                start=(hh == 0), stop=(hh == 1))
                        for b2 in range(2):
                            b = bb * BBLK + nn * 2 + b2
                            nc.vector.scalar_tensor_tensor(
                                out=zjunk, in0=zps[:, b2 * T:(b2 + 1) * T],
                                scalar=1.0, op0=ALU.mult,
                                in1=ursb, op1=ALU.mult,
                                accum_out=acc[:, b:b + 1])
                nc.vector.tensor_scalar(
                    out=accb, in0=acc, scalar1=brusb[:, 0:1], scalar2=None,
                    op0=ALU.add)
                nc.sync.dma_start(out=out_d, in_=accb)
                nc.sync.dma_start(out=flag_d, in_=par[0:1, 0:2])

    nc.compile()
    return nc


_NC_CACHE = {}


def get_nc():
    if "fast" not in _NC_CACHE:
        _NC_CACHE["fast"] = build_nc()
    return _NC_CACHE["fast"]


def get_nc_slow():
    if "slow" not in _NC_CACHE:
        _NC_CACHE["slow"] = build_nc_slow()
    return _NC_CACHE["slow"]


def prep_inputs(x, W1, b1, tau_n1, tau_m1, W2, b2, tau_n2, tau_m2,
                Wr, br, tau_mr, warmup):
    """Host-side: per-core input dicts for the fast bass kernel, plus the
    host-verified layer-2/readout constants. Returns (in_maps, fast_ok)."""
    w = int(np.asarray(warmup))
    beta1 = _sig(tau_n1).reshape(NF)          # [H,K], j = h*4+k order
    alpha1 = _sig(tau_m1)                     # [H]
    beta2 = _sig(tau_n2).reshape(NF)
    alpha2 = _sig(tau_m2)
    alphar = _sig(tau_mr)                     # [O]

    g1 = (1.0 - beta1) * np.repeat(1.0 - alpha1, K)

    # fp8 weights, prescaled by WSC; row 700 = bias, rows 701.. = 0
    w1t = np.zeros((IC * 128, NF), np.float64)
    w1t[:IN] = np.asarray(W1, np.float64).T * WSC
    w1t[IN] = np.asarray(b1, np.float64) * WSC
    w1q = np.empty((NPR, 128, 2 * NF), ml_dtypes.float8_e4m3)
    for pr in range(NPR):
        w1q[pr, :, :NF] = w1t[2 * pr * 128:(2 * pr + 1) * 128]
        w1q[pr, :, NF:] = w1t[(2 * pr + 1) * 128:(2 * pr + 2) * 128]

    # selector: g/WSC weights, [128, 32] blocks per feature chunk, packed
    selm = np.zeros((128, NCF * 32), ml_dtypes.bfloat16)
    for cf in range(NCF):
        j = cf * 128 + np.arange(128)
        selm[np.arange(128), cf * 32 + np.arange(128) // 4] = g1[j] / WSC

    betacol = np.ascontiguousarray(
        beta1.reshape(NCF, 128).T[:, 0:4].astype(np.float32))
    # beta^RBLK slab for the certificate half (cf 4-7), zero at block 0 of
    # each (chunk, batch) stream
    brs = np.zeros((128, 4, BBLK, NBLK), np.float64)
    brs[:] = (beta1.reshape(NCF, 128).T[:, 4:8] ** RBLK)[:, :, None, None]
    brs[:, :, :, 0] = 0.0
    brsl = brs.reshape(128, NCC).astype(ml_dtypes.bfloat16)

    # host-exact layer-2 bias trajectory (valid when layer 1 has no spikes)
    b2g = np.asarray(b2, np.float64) * (1.0 - beta2)
    dtraj = np.zeros(NF)
    mtraj = np.zeros(H)
    m2max = -np.inf
    for _ in range(T):
        dtraj = beta2 * dtraj + b2g
        mtraj = alpha2 * mtraj + (1.0 - alpha2) * dtraj.reshape(H, K).sum(-1)
        m2max = max(m2max, mtraj.max())
    fast_ok = bool(m2max <= VTH - 0.05)

    # host-exact readout constant (valid when layer 2 has no spikes)
    mr = np.zeros(O)
    accr = np.zeros(O)
    for t_ in range(T):
        mr = mr * alphar + (1.0 - alphar) * np.asarray(br, np.float64)
        if t_ >= w:
            accr += mr
    outc = np.tile((accr / (T - w)).astype(np.float32)[:, None], (1, BL))

    xq_full = np.zeros((IC * 128, B, T), ml_dtypes.float8_e4m3)
    xq_full[:IN] = np.asarray(x).transpose(2, 0, 1)
    xq_full[IN] = 1.0
    # pair-interleaved: [NPR, 128, 2, B, T]
    xq_full = np.ascontiguousarray(
        xq_full.reshape(NPR, 2, 128, B, T).transpose(0, 2, 1, 3, 4))

    shared = dict(w1q=w1q, selm=selm, betacol=betacol, brsl=brsl, outc=outc)
    in_maps = []
    for c in range(N_CORES):
        m = dict(shared)
        m["xq"] = np.ascontiguousarray(
            xq_full[:, :, :, c * BL:(c + 1) * BL, :])
        in_maps.append(m)
    return in_maps, fast_ok


def prep_inputs_slow(x, W1, b1, tau_n1, tau_m1, W2, b2, tau_n2, tau_m2,
                     Wr, br, tau_mr, warmup):
    """Host-side prep for the general fallback kernel."""
    w = int(np.asarray(warmup))
    beta1 = _sig(tau_n1).reshape(NF)
    alpha1 = _sig(tau_m1)
    beta2 = _sig(tau_n2).reshape(NF)
    alpha2 = _sig(tau_m2)
    alphar = _sig(tau_mr)

    g1 = (1.0 - beta1) * np.repeat(1.0 - alpha1, K)
    g2 = (1.0 - beta2) * np.repeat(1.0 - alpha2, K)

    w1p = np.zeros((IC * 128, NF), np.float64)
    w1p[:IN] = np.asarray(W1, np.float64).T * g1
    w1p[IN] = np.asarray(b1, np.float64) * g1
    w1p = w1p.astype(ml_dtypes.bfloat16)

    w2p = (np.asarray(W2, np.float64).T * g2).astype(ml_dtypes.bfloat16)
    b2g = np.asarray(b2, np.float64) * g2
    dtraj = np.zeros(NF)
    mh2b = np.zeros((H, T))
    mtraj = np.zeros(H)
    for t_ in range(T):
        dtraj = _sig(tau_n2).reshape(NF) * dtraj + b2g
        mtraj = _sig(tau_m2) * mtraj + dtraj.reshape(H, K).sum(-1)
        mh2b[:, t_] = mtraj
    mh2b_dev = np.zeros((128, 2 * T), np.float64)
    mh2b_dev[:, :T] = mh2b[:128]
    mh2b_dev[:, T:] = mh2b[128:]
    mh2b_dev = mh2b_dev.astype(ml_dtypes.bfloat16)

    wrt = np.zeros((128, 2 * O), np.float64)
    wrt[:, :O] = np.asarray(Wr, np.float64).T[:128]
    wrt[:, O:] = np.asarray(Wr, np.float64).T[128:]
    wrt = wrt.astype(ml_dtypes.bfloat16)

    def bslab(beta):
        s = np.tile(beta.reshape(NCF, 128, 1).astype(ml_dtypes.bfloat16),
                    (1, 1, NSL))
        s.reshape(NCF, 128, BBLK, T)[:, :, :, 0] = 0.0
        return s

    bsl1 = bslab(beta1)
    bsl2 = bslab(beta2)

    def aslab(alpha):
        a2 = alpha.reshape(2, 128).astype(ml_dtypes.bfloat16)
        s = np.tile(a2[:, :, None], (1, 1, NSL))
        s.reshape(2, 128, BBLK, T)[:, :, :, 0] = 0.0
        return s

    asl = np.concatenate([aslab(alpha1), aslab(alpha2)], axis=0)
    asl = asl.transpose(1, 0, 2).reshape(128, 4 * NSL).copy()

    acol = np.stack([alpha1[:128], alpha1[128:], alpha2[:128], alpha2[128:]],
                    axis=1).astype(np.float32)

    selm = np.zeros((128, 32), ml_dtypes.bfloat16)
    selm[np.arange(128), np.arange(128) // 4] = 1.0

    tt = np.arange(T, dtype=np.float64)[:, None]
    ar = alphar[None, :]
    u = ar ** np.maximum(0, w - tt) - ar ** (T - tt)
    ur = (u.T / (T - w)).astype(np.float32)
    bru = (np.asarray(br, np.float64) * u.sum(0) / (T - w)) \
        .astype(np.float32)[:, None]

    xt_full = np.zeros((IC * 128, B, T), ml_dtypes.bfloat16)
    xt_full[:IN] = np.asarray(x).transpose(2, 0, 1)
    xt_full[IN] = 1.0

    shared = dict(w1p=w1p, w2p=w2p, mh2b=mh2b_dev, wrt=wrt,
                  bsl1=bsl1, bsl2=bsl2, asl=asl, acol=acol, selm=selm,
                  ur=ur, bru=bru)
    in_maps = []
    for c in range(N_CORES):
        m = dict(shared)
        m["xt"] = np.ascontiguousarray(xt_full[:, c * BL:(c + 1) * BL, :])
        in_maps.append(m)
    return in_maps


def _run_slow(**inputs):
    in_maps = prep_inputs_slow(**inputs)
    res = bass_utils.run_bass_kernel_spmd(
        get_nc_slow(), in_maps, core_ids=list(range(N_CORES)))
    out = np.empty((B, O), np.float32)
    for c in range(N_CORES):
        out[c * BL:(c + 1) * BL] = res.results[c]["out"].T
    return out


def kernel(**inputs):
    in_maps, fast_ok = prep_inputs(**inputs)
    if not fast_ok:
        return _run_slow(**inputs)
    res = bass_utils.run_bass_kernel_spmd(
        get_nc(), in_maps, core_ids=list(range(N_CORES)))
    if any(r["flag"].sum() > 0 for r in res.results):
        # certificate failed: spikes may exist, use the general kernel
        return _run_slow(**inputs)
    out = np.empty((B, O), np.float32)
    for c in range(N_CORES):
        out[c * BL:(c + 1) * BL] = res.results[c]["out"].T
    return out
